# revision 21
# baseline (speedup 1.0000x reference)
"""Multi-head causal attention (B=4, S=2048, D=1024, H=16) on 8 trn2 NeuronCores.

Sharding: data-parallel over batch (4) x tensor-parallel over heads (2 groups
of 8 heads).  Core c handles batch c//2, head-group c%2.  Each core computes
its 512-wide slice of Q/K/V, causal attention for its 8 heads, and a partial
out-projection (row-parallel Wo).  The host sums the two partials per batch
and adds the bias (the "all-reduce" of the row-parallel out_proj).

Kernel layout notes (per core):
 - x arrives pre-transposed (and pre-cast to bf16) from host as xt
   [1024, 2048] so the contraction dim (d_in) is on partitions for all
   projection matmuls.
 - Q^T, K^T stored [d'=128 (2 heads), s] in bf16: directly usable as
   scores-matmul operands (S^T[k,q] = K^T_tile.T @ Q^T) with d on partitions.
 - V stored naturally [s, d'] with a ones-column appended per head (65-wide
   head slots) so the ctx matmul also produces the softmax denominators.
 - Scores are computed transposed (S^T: k on partitions, q free).  Softmax
   needs no max-stabilization (scores ~ N(0,1) after the 1/8 scale).  Causal
   masking: the fully-masked strip of a diagonal tile is memset to 0, the
   128x128 boundary block is multiplied by a precomputed triangular bf16
   mask, and only the live strip is exp'd.
 - Normalization is kept OFF the PE critical path: per head, the
   unnormalized ctx^T and the denominator row are copied out of PSUM by DVE
   (freeing the PSUM accumulator early); the reciprocal (DVE approx-fast,
   SBUF source only -- PSUM source is broken on HW), the K=1 broadcast
   matmuls, and the final normalize-multiplies are all batched at the end
   of the q-block where they pipeline behind other work.
 - The PE instruction queue is in-order, so phase overlap must be done at
   emission time: the projection matmuls for q-block n+1 are emitted as
   small "filler" quanta interleaved between the attention batches of
   q-block n.  While attention waits on ScalarE exp, PE executes projection
   quanta instead of idling (also keeping the HAM clock-gate warm).
"""

import numpy as np

import concourse.bacc as bacc
import concourse.mybir as mybir
from concourse import tile
from concourse.bass_utils import run_bass_kernel_spmd

F32 = mybir.dt.float32
BF16 = mybir.dt.bfloat16
EXP = mybir.ActivationFunctionType.Exp

B, S, DIN, DOUT, H = 4, 2048, 1024, 1024, 16
NCORES = 8
DG = 512          # d_out slice per core (8 heads)
NH = 8            # heads per core
HD = 64
NKT = DIN // 128  # 8 contraction tiles for projections
NQB = S // 512    # 4 q blocks of 512
NKB = S // 128    # 16 k blocks of 128
NDB = DG // 128   # 4 d'-blocks of 128 (2 heads each)

NP_BF16 = mybir.dt.np(BF16)

LAST_EXEC_TIME_NS = None


def build_nc():
    nc = bacc.Bacc()
    xt = nc.dram_tensor("xt", [DIN, S], BF16, kind="ExternalInput")
    wq = nc.dram_tensor("wq", [DIN, DG], BF16, kind="ExternalInput")
    wk = nc.dram_tensor("wk", [DIN, DG], BF16, kind="ExternalInput")
    wv = nc.dram_tensor("wv", [DIN, DG], BF16, kind="ExternalInput")
    wo = nc.dram_tensor("wo", [DG, DOUT], BF16, kind="ExternalInput")
    out = nc.dram_tensor("out", [S, DOUT], F32, kind="ExternalOutput")

    with tile.TileContext(nc) as tc:
        with (
            tc.tile_pool(name="persist", bufs=1) as persist,
            tc.tile_pool(name="xt", bufs=2) as xt_pool,
            tc.tile_pool(name="eb", bufs=4) as e_pool,
            tc.tile_pool(name="rp", bufs=2) as r_pool,
            tc.tile_pool(name="cu", bufs=9) as cu_pool,
            tc.tile_pool(name="ob", bufs=3) as o_pool,
            tc.tile_pool(name="psA", bufs=3, space="PSUM") as psA,
            tc.tile_pool(name="psC", bufs=2, space="PSUM") as psC,
        ):
            # ---- persistent SBUF tensors ----
            wq_sb = persist.tile([128, NKT, DG], BF16)
            wk_sb = persist.tile([128, NKT, DG], BF16)
            wv_sb = persist.tile([128, NKT, DG], BF16)
            wo_sb = persist.tile([128, NDB, DOUT], BF16)
            qt_sb = persist.tile([128, NDB, S], BF16)
            kt_sb = persist.tile([128, NDB, S], BF16)
            v_sb = persist.tile([128, NKB, NH, HD + 1], BF16)
            ct_sb = persist.tile([128, NDB, S], BF16)
            mask_sb = persist.tile([128, 128], BF16)
            ones_sb = persist.tile([1, 64], BF16)

            # ---- one-time setup ----
            nc.vector.memset(ones_sb[:], 1.0)
            nc.vector.memset(v_sb[:, :, :, HD : HD + 1], 1.0)
            nc.vector.memset(mask_sb[:], 1.0)
            # triangular causal boundary block: keep where q_local >= k_local
            nc.gpsimd.affine_select(
                out=mask_sb[:],
                in_=mask_sb[:],
                pattern=[[1, 128]],
                base=0,
                channel_multiplier=-1,
                compare_op=mybir.AluOpType.is_ge,
                fill=0.0,
            )

            xt_r = xt.rearrange("(kt p) s -> p kt s", p=128)
            xt_tiles = [None] * NQB

            def load_xt(n):
                t = xt_pool.tile([128, NKT, 512], BF16, tag="xt")
                nc.sync.dma_start(out=t[:], in_=xt_r[:, :, n * 512 : (n + 1) * 512])
                xt_tiles[n] = t

            # first x block before the weights so phase A(0) starts ASAP
            load_xt(0)
            for w_dram, w_sb in ((wq, wq_sb), (wk, wk_sb), (wv, wv_sb)):
                w_r = w_dram.rearrange("(kt p) d -> p kt d", p=128)
                for kt in range(NKT):
                    nc.sync.dma_start(out=w_sb[:, kt, :], in_=w_r[:, kt, :])
            wo_r = wo.rearrange("(t p) e -> p t e", p=128)
            for p in range(NDB):
                nc.sync.dma_start(out=wo_sb[:, p, :], in_=wo_r[:, p, :])

            def phase_a_quanta(n):
                """Emit projections for s-block n as a list of small closures.

                Each quantum is ~2 matmuls (or one PSUM->SBUF copy) so it can
                be interleaved between attention batches as PE filler.
                """
                quanta = []
                xt_t = xt_tiles[n]
                state = {}

                def q_group(w_sb, dst, mp):
                    def alloc():
                        state[("ps", w_sb.name, mp)] = psA.tile(
                            [128, 1024], F32, tag="ps", name=f"psa_{n}_{w_sb.name}_{mp}"
                        )

                    quanta.append(alloc)
                    for m01 in range(2):
                        m = mp * 2 + m01
                        for kt0 in range(0, NKT, 2):

                            def mm2(m01=m01, m=m, kt0=kt0, w_sb=w_sb, mp=mp):
                                ps = state[("ps", w_sb.name, mp)]
                                for kt in (kt0, kt0 + 1):
                                    nc.tensor.matmul(
                                        ps[:, m01 * 512 : (m01 + 1) * 512],
                                        lhsT=w_sb[:, kt, m * 128 : (m + 1) * 128],
                                        rhs=xt_t[:, kt, :],
                                        start=(kt == 0),
                                        stop=(kt == NKT - 1),
                                    )

                            quanta.append(mm2)

                    def cp(w_sb=w_sb, dst=dst, mp=mp):
                        ps = state[("ps", w_sb.name, mp)]
                        nc.vector.tensor_copy(
                            dst[:, mp * 2 : mp * 2 + 2, n * 512 : (n + 1) * 512],
                            ps.rearrange("p (m s) -> p m s", m=2),
                        )

                    quanta.append(cp)

                for w_sb, dst in ((wq_sb, qt_sb), (wk_sb, kt_sb)):
                    for mp in range(2):
                        q_group(w_sb, dst, mp)

                def v_group(sp):
                    def alloc(sp=sp):
                        state[("psv", sp)] = psA.tile([128, 1024], F32, tag="ps", name=f"psv_{n}_{sp}")

                    quanta.append(alloc)
                    for s01 in range(2):
                        ss = sp * 2 + s01
                        for kt0 in range(0, NKT, 2):

                            def mm2(s01=s01, ss=ss, kt0=kt0, sp=sp):
                                ps = state[("psv", sp)]
                                for kt in (kt0, kt0 + 1):
                                    nc.tensor.matmul(
                                        ps[:, s01 * 512 : (s01 + 1) * 512],
                                        lhsT=xt_t[:, kt, ss * 128 : (ss + 1) * 128],
                                        rhs=wv_sb[:, kt, :],
                                        start=(kt == 0),
                                        stop=(kt == NKT - 1),
                                    )

                            quanta.append(mm2)

                    def cp(sp=sp):
                        ps = state[("psv", sp)]
                        gss = n * 4 + sp * 2
                        nc.vector.tensor_copy(
                            v_sb[:, gss : gss + 2, :, 0:HD],
                            ps.rearrange("p (u h e) -> p u h e", u=2, e=HD),
                        )

                    quanta.append(cp)

                for sp in range(2):
                    v_group(sp)
                return quanta

            def phase_b(j, filler):
                """Attention for q-block j, heads processed in even/odd pairs
                whose 64-row scores matmuls occupy disjoint PE row-groups and
                run concurrently.  `filler` quanta (next block's projections,
                previous block's out-projection) are drained between batches
                to keep PE busy during exp waits."""
                nkb = 4 * j + 4
                nbatches = (NH // 2) * (nkb // 2)
                nq = len(filler)
                drained = 0
                bi = 0
                pending = []  # (h, cu, rc) awaiting broadcast+multiply

                def finish_norm(h, cu, rc):
                    dblk, poff = h // 2, (h % 2) * 64
                    pb = psA.tile([64, 512], F32, tag="ps", name=f"pb_{j}_{h}")
                    nc.tensor.matmul(
                        pb[:], lhsT=ones_sb[:], rhs=rc[:], start=True, stop=True
                    )
                    rb = r_pool.tile([64, 512], BF16, tag="rb", bufs=2)
                    nc.scalar.copy(rb[:], pb[:])
                    nc.vector.tensor_mul(
                        ct_sb[poff : poff + 64, dblk, j * 512 : (j + 1) * 512],
                        cu[:],
                        rb[:],
                    )

                for hp in range(NH // 2):
                    he, ho = 2 * hp, 2 * hp + 1
                    dblk = hp
                    # finish the previous pair's normalization now: their
                    # reciprocals are long since ready, so the broadcast
                    # matmuls slot in without stalling PE
                    for ent in pending:
                        finish_norm(*ent)
                    pending = []
                    pcs = [
                        psC.tile([65, 512], F32, tag="pc", name=f"pc_{j}_{he}"),
                        psC.tile([65, 512], F32, tag="pc", name=f"pc_{j}_{ho}"),
                    ]
                    pss = [None, None]
                    ebs = [None, None]
                    for ib in range(nkb // 2):
                        for po2 in range(2):
                            pss[po2] = psA.tile(
                                [128, 1024], F32, tag="ps", name=f"ps_{j}_{hp}_{po2}"
                            )
                        for t in range(2):
                            i = 2 * ib + t
                            for po2 in range(2):  # adjacent -> row-concurrent
                                poff = po2 * 64
                                nc.tensor.matmul(
                                    pss[po2][:, t * 512 : (t + 1) * 512],
                                    lhsT=kt_sb[
                                        poff : poff + 64, dblk, i * 128 : (i + 1) * 128
                                    ],
                                    rhs=qt_sb[
                                        poff : poff + 64, dblk, j * 512 : (j + 1) * 512
                                    ],
                                    start=True,
                                    stop=True,
                                )
                        for po2 in range(2):
                            eb = e_pool.tile([128, 1024], BF16, tag="eb", name=f"eb_{j}_{hp}_{po2}")
                            ebs[po2] = eb
                            ps = pss[po2]
                            d1 = 2 * ib + 1 - 4 * j
                            if d1 < 0:
                                nc.scalar.activation(eb[:], ps[:], EXP, scale=0.125)
                            else:
                                for t in range(2):
                                    i = 2 * ib + t
                                    dd = i - 4 * j
                                    lo = t * 512
                                    if dd < 0:
                                        nc.scalar.activation(
                                            eb[:, lo : lo + 512],
                                            ps[:, lo : lo + 512],
                                            EXP,
                                            scale=0.125,
                                        )
                                    else:
                                        z = 128 * dd
                                        if z > 0:
                                            nc.vector.memset(eb[:, lo : lo + z], 0.0)
                                        nc.scalar.activation(
                                            eb[:, lo + z : lo + 512],
                                            ps[:, lo + z : lo + 512],
                                            EXP,
                                            scale=0.125,
                                        )
                                        nc.vector.tensor_mul(
                                            eb[:, lo + z : lo + z + 128],
                                            eb[:, lo + z : lo + z + 128],
                                            mask_sb[:],
                                        )
                        for t in range(2):
                            i = 2 * ib + t
                            for po2 in range(2):
                                nc.tensor.matmul(
                                    pcs[po2][:],
                                    lhsT=v_sb[:, i, 2 * hp + po2, :],
                                    rhs=ebs[po2][:, t * 512 : (t + 1) * 512],
                                    start=(i == 0),
                                    stop=(i == nkb - 1),
                                )
                        bi += 1
                        want = nq * bi // nbatches
                        while drained < want:
                            filler[drained]()
                            drained += 1
                    # pull ctx + denominator out of PSUM (DVE only), free pcs
                    for po2 in range(2):
                        h = 2 * hp + po2
                        pc = pcs[po2]
                        dn = r_pool.tile([1, 512], F32, tag="dn", bufs=3)
                        nc.vector.tensor_copy(dn[:], pc[64:65, :])
                        rc32 = r_pool.tile([1, 512], F32, tag="rc32", bufs=3)
                        nc.vector.reciprocal_approx_fast(rc32[:], dn[:])
                        rc = r_pool.tile([1, 512], BF16, tag="rc", bufs=4)
                        nc.vector.tensor_copy(rc[:], rc32[:])
                        cu = cu_pool.tile([64, 512], BF16, tag="cu")
                        nc.vector.tensor_copy(cu[:], pc[0:64, :])
                        pending.append((h, cu, rc))
                while drained < nq:
                    filler[drained]()
                    drained += 1
                for ent in pending:
                    finish_norm(*ent)

            def phase_c_quanta(n):
                quanta = []
                for qq in range(4 * n, 4 * n + 4):
                    for e2 in range(2):

                        def unit(qq=qq, e2=e2):
                            po = psA.tile(
                                [128, 512], F32, tag="ps", name=f"po_{qq}_{e2}"
                            )
                            for p in range(NDB):
                                nc.tensor.matmul(
                                    po[:],
                                    lhsT=ct_sb[:, p, qq * 128 : (qq + 1) * 128],
                                    rhs=wo_sb[:, p, e2 * 512 : (e2 + 1) * 512],
                                    start=(p == 0),
                                    stop=(p == NDB - 1),
                                )
                            ob = o_pool.tile([128, 512], F32, tag="ob", name=f"ob_{qq}_{e2}")
                            nc.vector.tensor_copy(ob[:], po[:])
                            nc.sync.dma_start(
                                out=out[
                                    qq * 128 : (qq + 1) * 128,
                                    e2 * 512 : (e2 + 1) * 512,
                                ],
                                in_=ob[:],
                            )

                        quanta.append(unit)
                return quanta

            # ---- main schedule ----
            # A(0) runs plain; B(n) is interleaved with A(n+1) + C(n-1).
            for q in phase_a_quanta(0):
                q()
            for n in range(NQB):
                filler = []
                if n >= 1:
                    filler += phase_c_quanta(n - 1)
                if n + 1 < NQB:
                    load_xt(n + 1)
                    filler += phase_a_quanta(n + 1)
                phase_b(n, filler)
            for q in phase_c_quanta(NQB - 1):
                q()
    nc.compile()
    return nc


_NC_CACHE = None


def _get_nc():
    global _NC_CACHE
    if _NC_CACHE is None:
        _NC_CACHE = build_nc()
    return _NC_CACHE


def make_in_maps(x, Wq, Wk, Wv, Wo):
    x = np.asarray(x, dtype=np.float32).astype(NP_BF16)
    Wq = np.asarray(Wq, dtype=np.float32).astype(NP_BF16)
    Wk = np.asarray(Wk, dtype=np.float32).astype(NP_BF16)
    Wv = np.asarray(Wv, dtype=np.float32).astype(NP_BF16)
    Wo = np.asarray(Wo, dtype=np.float32).astype(NP_BF16)
    in_maps = []
    for c in range(NCORES):
        b, g = c // 2, c % 2
        sl = slice(g * DG, (g + 1) * DG)
        in_maps.append(
            {
                "xt": np.ascontiguousarray(x[b].T),
                "wq": np.ascontiguousarray(Wq[:, sl]),
                "wk": np.ascontiguousarray(Wk[:, sl]),
                "wv": np.ascontiguousarray(Wv[:, sl]),
                "wo": np.ascontiguousarray(Wo[sl, :]),
            }
        )
    return in_maps


def _install_ntff_hook():
    """Shim antenv.axon_hooks (absent in this image) so trace=True works."""
    import sys
    import types

    try:
        import antenv.axon_hooks  # noqa: F401

        return
    except ImportError:
        pass
    try:
        import antenv
        from trn_agent_boot.trn_boot import _ntff_profile_via_ctypes

        hook = _ntff_profile_via_ctypes("/opt/axon/libaxon_pjrt.so")
        mod = types.ModuleType("antenv.axon_hooks")
        mod._hook = hook
        mod.get_axon_ntff_profile_hook = lambda: mod._hook
        mod.set_axon_ntff_profile_hook = lambda h: setattr(mod, "_hook", h)
        sys.modules["antenv.axon_hooks"] = mod
        antenv.axon_hooks = mod
    except Exception as e:  # degrade to no-trace
        print("ntff hook shim failed:", e)


def kernel(x, Wq, Wk, Wv, Wo, bo, _trace=False):
    global LAST_EXEC_TIME_NS
    if _trace:
        _install_ntff_hook()
    bo = np.asarray(bo, dtype=np.float32)
    nc = _get_nc()
    in_maps = make_in_maps(x, Wq, Wk, Wv, Wo)
    res = run_bass_kernel_spmd(nc, in_maps, list(range(NCORES)), trace=_trace)
    LAST_EXEC_TIME_NS = res.exec_time_ns
    out = np.empty((B, S, DOUT), dtype=np.float32)
    for b in range(B):
        out[b] = res.results[2 * b]["out"] + res.results[2 * b + 1]["out"] + bo
    return out


# revision 22
# speedup vs baseline: 1.0798x; 1.0798x over previous
"""Multi-head causal attention (B=4, S=2048, D=1024, H=16) on 8 trn2 NeuronCores.

Sharding: data-parallel over batch (4) x tensor-parallel over heads (2 groups
of 8 heads).  Core c handles batch c//2, head-group c%2.  Each core computes
its 512-wide slice of Q/K/V, causal attention for its 8 heads, and a partial
out-projection (row-parallel Wo).  The host sums the two partials per batch
and adds the bias (the "all-reduce" of the row-parallel out_proj).

Kernel layout notes (per core):
 - x arrives pre-transposed (and pre-cast to bf16) from host as xt
   [1024, 2048] so the contraction dim (d_in) is on partitions for all
   projection matmuls.
 - Q^T, K^T stored [d'=128 (2 heads), s] in bf16: directly usable as
   scores-matmul operands (S^T[k,q] = K^T_tile.T @ Q^T) with d on partitions.
 - V stored naturally [s, d'] with a ones-column appended per head (65-wide
   head slots) so the ctx matmul also produces the softmax denominators.
 - Scores are computed transposed (S^T: k on partitions, q free).  Softmax
   needs no max-stabilization (scores ~ N(0,1) after the 1/8 scale).  Causal
   masking: the fully-masked strip of a diagonal tile is memset to 0, the
   128x128 boundary block is multiplied by a precomputed triangular bf16
   mask, and only the live strip is exp'd.
 - Normalization is kept OFF the PE critical path: per head, the
   unnormalized ctx^T and the denominator row are copied out of PSUM by DVE
   (freeing the PSUM accumulator early); the reciprocal (DVE approx-fast,
   SBUF source only -- PSUM source is broken on HW), the K=1 broadcast
   matmuls, and the final normalize-multiplies are all batched at the end
   of the q-block where they pipeline behind other work.
 - The PE instruction queue is in-order, so phase overlap must be done at
   emission time: the projection matmuls for q-block n+1 are emitted as
   small "filler" quanta interleaved between the attention batches of
   q-block n.  While attention waits on ScalarE exp, PE executes projection
   quanta instead of idling (also keeping the HAM clock-gate warm).
"""

import numpy as np

import concourse.bacc as bacc
import concourse.mybir as mybir
from concourse import tile
from concourse.bass_utils import run_bass_kernel_spmd

F32 = mybir.dt.float32
BF16 = mybir.dt.bfloat16
EXP = mybir.ActivationFunctionType.Exp

B, S, DIN, DOUT, H = 4, 2048, 1024, 1024, 16
NCORES = 8
DG = 512          # d_out slice per core (8 heads)
NH = 8            # heads per core
HD = 64
NKT = DIN // 128  # 8 contraction tiles for projections
NQB = S // 512    # 4 q blocks of 512
NKB = S // 128    # 16 k blocks of 128
NDB = DG // 128   # 4 d'-blocks of 128 (2 heads each)

NP_BF16 = mybir.dt.np(BF16)

LAST_EXEC_TIME_NS = None


def build_nc():
    nc = bacc.Bacc()
    xt = nc.dram_tensor("xt", [DIN, S], BF16, kind="ExternalInput")
    wq = nc.dram_tensor("wq", [DIN, DG], BF16, kind="ExternalInput")
    wk = nc.dram_tensor("wk", [DIN, DG], BF16, kind="ExternalInput")
    wv = nc.dram_tensor("wv", [DIN, DG], BF16, kind="ExternalInput")
    wo = nc.dram_tensor("wo", [DG, DOUT], BF16, kind="ExternalInput")
    out = nc.dram_tensor("out", [S, DOUT], F32, kind="ExternalOutput")

    with tile.TileContext(nc) as tc:
        with (
            tc.tile_pool(name="persist", bufs=1) as persist,
            tc.tile_pool(name="xt", bufs=2) as xt_pool,
            tc.tile_pool(name="eb", bufs=4) as e_pool,
            tc.tile_pool(name="rp", bufs=2) as r_pool,
            tc.tile_pool(name="cu", bufs=9) as cu_pool,
            tc.tile_pool(name="ob", bufs=3) as o_pool,
            tc.tile_pool(name="psA", bufs=3, space="PSUM") as psA,
            tc.tile_pool(name="psC", bufs=2, space="PSUM") as psC,
        ):
            # ---- persistent SBUF tensors ----
            wq_sb = persist.tile([128, NKT, DG], BF16)
            wk_sb = persist.tile([128, NKT, DG], BF16)
            wv_sb = persist.tile([128, NKT, DG], BF16)
            wo_sb = persist.tile([128, NDB, DOUT], BF16)
            qt_sb = persist.tile([128, NDB, S], BF16)
            kt_sb = persist.tile([128, NDB, S], BF16)
            v_sb = persist.tile([128, NKB, NH, HD + 1], BF16)
            ct_sb = persist.tile([128, NDB, S], BF16)
            mask_sb = persist.tile([128, 128], BF16)
            ones_sb = persist.tile([1, 64], BF16)

            # ---- one-time setup ----
            nc.vector.memset(ones_sb[:], 1.0)
            nc.vector.memset(v_sb[:, :, :, HD : HD + 1], 1.0)
            nc.vector.memset(mask_sb[:], 1.0)
            # triangular causal boundary block: keep where q_local >= k_local
            nc.gpsimd.affine_select(
                out=mask_sb[:],
                in_=mask_sb[:],
                pattern=[[1, 128]],
                base=0,
                channel_multiplier=-1,
                compare_op=mybir.AluOpType.is_ge,
                fill=0.0,
            )

            xt_r = xt.rearrange("(kt p) s -> p kt s", p=128)
            xt_tiles = [None] * NQB

            def load_xt(n):
                t = xt_pool.tile([128, NKT, 512], BF16, tag="xt")
                nc.sync.dma_start(out=t[:], in_=xt_r[:, :, n * 512 : (n + 1) * 512])
                xt_tiles[n] = t

            # first x block before the weights so phase A(0) starts ASAP
            load_xt(0)
            for w_dram, w_sb in ((wq, wq_sb), (wk, wk_sb), (wv, wv_sb)):
                w_r = w_dram.rearrange("(kt p) d -> p kt d", p=128)
                for kt in range(NKT):
                    nc.sync.dma_start(out=w_sb[:, kt, :], in_=w_r[:, kt, :])
            wo_r = wo.rearrange("(t p) e -> p t e", p=128)
            for p in range(NDB):
                nc.sync.dma_start(out=wo_sb[:, p, :], in_=wo_r[:, p, :])

            def phase_a_quanta(n):
                """Emit projections for s-block n as a list of small closures.

                Each quantum is ~2 matmuls (or one PSUM->SBUF copy) so it can
                be interleaved between attention batches as PE filler.
                """
                quanta = []
                xt_t = xt_tiles[n]
                state = {}

                def q_group(w_sb, dst, mp):
                    def alloc():
                        state[("ps", w_sb.name, mp)] = psA.tile(
                            [128, 1024], F32, tag="ps", name=f"psa_{n}_{w_sb.name}_{mp}"
                        )

                    quanta.append(alloc)
                    for m01 in range(2):
                        m = mp * 2 + m01
                        for kt0 in range(0, NKT, 2):

                            def mm2(m01=m01, m=m, kt0=kt0, w_sb=w_sb, mp=mp):
                                ps = state[("ps", w_sb.name, mp)]
                                for kt in (kt0, kt0 + 1):
                                    nc.tensor.matmul(
                                        ps[:, m01 * 512 : (m01 + 1) * 512],
                                        lhsT=w_sb[:, kt, m * 128 : (m + 1) * 128],
                                        rhs=xt_t[:, kt, :],
                                        start=(kt == 0),
                                        stop=(kt == NKT - 1),
                                    )

                            quanta.append(mm2)

                    def cp(w_sb=w_sb, dst=dst, mp=mp):
                        ps = state[("ps", w_sb.name, mp)]
                        nc.vector.tensor_copy(
                            dst[:, mp * 2 : mp * 2 + 2, n * 512 : (n + 1) * 512],
                            ps.rearrange("p (m s) -> p m s", m=2),
                        )

                    quanta.append(cp)

                for w_sb, dst in ((wq_sb, qt_sb), (wk_sb, kt_sb)):
                    for mp in range(2):
                        q_group(w_sb, dst, mp)

                def v_group(sp):
                    def alloc(sp=sp):
                        state[("psv", sp)] = psA.tile([128, 1024], F32, tag="ps", name=f"psv_{n}_{sp}")

                    quanta.append(alloc)
                    for s01 in range(2):
                        ss = sp * 2 + s01
                        for kt0 in range(0, NKT, 2):

                            def mm2(s01=s01, ss=ss, kt0=kt0, sp=sp):
                                ps = state[("psv", sp)]
                                for kt in (kt0, kt0 + 1):
                                    nc.tensor.matmul(
                                        ps[:, s01 * 512 : (s01 + 1) * 512],
                                        lhsT=xt_t[:, kt, ss * 128 : (ss + 1) * 128],
                                        rhs=wv_sb[:, kt, :],
                                        start=(kt == 0),
                                        stop=(kt == NKT - 1),
                                    )

                            quanta.append(mm2)

                    def cp(sp=sp):
                        ps = state[("psv", sp)]
                        gss = n * 4 + sp * 2
                        nc.vector.tensor_copy(
                            v_sb[:, gss : gss + 2, :, 0:HD],
                            ps.rearrange("p (u h e) -> p u h e", u=2, e=HD),
                        )

                    quanta.append(cp)

                for sp in range(2):
                    v_group(sp)
                return quanta

            def phase_b(j, filler):
                """Attention for q-block j.  Heads run in even/odd pairs: each
                batch computes ONE k-tile for BOTH heads into one psum tile
                (even head rows 0-63, odd head rows 64-127 of the PE array ->
                the two 64-row scores matmuls execute concurrently), and one
                exp covers both.  `filler` quanta are drained between batches
                to keep PE busy during exp waits."""
                nkb = 4 * j + 4
                nbatches = (NH // 2) * nkb
                nq = len(filler)
                drained = 0
                bi = 0
                pending = []  # (h, cu, rc) awaiting broadcast+multiply

                def finish_norm(h, cu, rc):
                    dblk, poff = h // 2, (h % 2) * 64
                    pb = psA.tile([64, 512], F32, tag="ps", name=f"pb_{j}_{h}")
                    nc.tensor.matmul(
                        pb[:], lhsT=ones_sb[:], rhs=rc[:], start=True, stop=True
                    )
                    rb = r_pool.tile([64, 512], BF16, tag="rb", bufs=2)
                    nc.scalar.copy(rb[:], pb[:])
                    nc.vector.tensor_mul(
                        ct_sb[poff : poff + 64, dblk, j * 512 : (j + 1) * 512],
                        cu[:],
                        rb[:],
                    )

                for hp in range(NH // 2):
                    dblk = hp
                    for ent in pending:
                        finish_norm(*ent)
                    pending = []
                    pcs = [
                        psC.tile([65, 512], F32, tag="pc", name=f"pc_{j}_{2 * hp}"),
                        psC.tile([65, 512], F32, tag="pc", name=f"pc_{j}_{2 * hp + 1}"),
                    ]
                    for i in range(nkb):
                        ps = psA.tile([128, 1024], F32, tag="ps", name=f"ps_{j}_{hp}_{i}")
                        for po2 in range(2):  # adjacent -> row-concurrent
                            poff = po2 * 64
                            nc.tensor.matmul(
                                ps[:, po2 * 512 : (po2 + 1) * 512],
                                lhsT=kt_sb[
                                    poff : poff + 64, dblk, i * 128 : (i + 1) * 128
                                ],
                                rhs=qt_sb[
                                    poff : poff + 64, dblk, j * 512 : (j + 1) * 512
                                ],
                                start=True,
                                stop=True,
                            )
                        eb = e_pool.tile([128, 1024], BF16, tag="eb", name=f"eb_{j}_{hp}_{i}")
                        dd = i - 4 * j
                        if dd < 0:
                            nc.scalar.activation(eb[:], ps[:], EXP, scale=0.125)
                        else:
                            z = 128 * dd
                            for po2 in range(2):
                                lo = po2 * 512
                                if z > 0:
                                    nc.vector.memset(eb[:, lo : lo + z], 0.0)
                                nc.scalar.activation(
                                    eb[:, lo + z : lo + 512],
                                    ps[:, lo + z : lo + 512],
                                    EXP,
                                    scale=0.125,
                                )
                                nc.vector.tensor_mul(
                                    eb[:, lo + z : lo + z + 128],
                                    eb[:, lo + z : lo + z + 128],
                                    mask_sb[:],
                                )
                        for po2 in range(2):
                            nc.tensor.matmul(
                                pcs[po2][:],
                                lhsT=v_sb[:, i, 2 * hp + po2, :],
                                rhs=eb[:, po2 * 512 : (po2 + 1) * 512],
                                start=(i == 0),
                                stop=(i == nkb - 1),
                            )
                        bi += 1
                        want = nq * bi // nbatches
                        while drained < want:
                            filler[drained]()
                            drained += 1
                    # pull ctx + denominator out of PSUM (DVE only), free pcs
                    for po2 in range(2):
                        h = 2 * hp + po2
                        pc = pcs[po2]
                        dn = r_pool.tile([1, 512], F32, tag="dn", bufs=3)
                        nc.vector.tensor_copy(dn[:], pc[64:65, :])
                        rc32 = r_pool.tile([1, 512], F32, tag="rc32", bufs=3)
                        nc.vector.reciprocal_approx_fast(rc32[:], dn[:])
                        rc = r_pool.tile([1, 512], BF16, tag="rc", bufs=4)
                        nc.vector.tensor_copy(rc[:], rc32[:])
                        cu = cu_pool.tile([64, 512], BF16, tag="cu")
                        nc.vector.tensor_copy(cu[:], pc[0:64, :])
                        pending.append((h, cu, rc))
                while drained < nq:
                    filler[drained]()
                    drained += 1
                for ent in pending:
                    finish_norm(*ent)

            def phase_c_quanta(n):
                quanta = []
                for qq in range(4 * n, 4 * n + 4):
                    for e2 in range(2):

                        def unit(qq=qq, e2=e2):
                            po = psA.tile(
                                [128, 512], F32, tag="ps", name=f"po_{qq}_{e2}"
                            )
                            for p in range(NDB):
                                nc.tensor.matmul(
                                    po[:],
                                    lhsT=ct_sb[:, p, qq * 128 : (qq + 1) * 128],
                                    rhs=wo_sb[:, p, e2 * 512 : (e2 + 1) * 512],
                                    start=(p == 0),
                                    stop=(p == NDB - 1),
                                )
                            ob = o_pool.tile(
                                [128, 512], F32, tag="ob", name=f"ob_{qq}_{e2}"
                            )
                            nc.vector.tensor_copy(ob[:], po[:])
                            nc.sync.dma_start(
                                out=out[
                                    qq * 128 : (qq + 1) * 128,
                                    e2 * 512 : (e2 + 1) * 512,
                                ],
                                in_=ob[:],
                            )

                        quanta.append(unit)
                return quanta

            # ---- main schedule ----
            # A(0) runs plain; B(n) is interleaved with A(n+1) + C(n-1).
            for q in phase_a_quanta(0):
                q()
            for n in range(NQB):
                filler = []
                if n >= 1:
                    filler += phase_c_quanta(n - 1)
                if n + 1 < NQB:
                    load_xt(n + 1)
                    filler += phase_a_quanta(n + 1)
                phase_b(n, filler)
            for q in phase_c_quanta(NQB - 1):
                q()
    nc.compile()
    return nc


_NC_CACHE = None


def _get_nc():
    global _NC_CACHE
    if _NC_CACHE is None:
        _NC_CACHE = build_nc()
    return _NC_CACHE


def make_in_maps(x, Wq, Wk, Wv, Wo):
    x = np.asarray(x, dtype=np.float32).astype(NP_BF16)
    Wq = np.asarray(Wq, dtype=np.float32).astype(NP_BF16)
    Wk = np.asarray(Wk, dtype=np.float32).astype(NP_BF16)
    Wv = np.asarray(Wv, dtype=np.float32).astype(NP_BF16)
    Wo = np.asarray(Wo, dtype=np.float32).astype(NP_BF16)
    in_maps = []
    for c in range(NCORES):
        b, g = c // 2, c % 2
        sl = slice(g * DG, (g + 1) * DG)
        in_maps.append(
            {
                "xt": np.ascontiguousarray(x[b].T),
                "wq": np.ascontiguousarray(Wq[:, sl]),
                "wk": np.ascontiguousarray(Wk[:, sl]),
                "wv": np.ascontiguousarray(Wv[:, sl]),
                "wo": np.ascontiguousarray(Wo[sl, :]),
            }
        )
    return in_maps


def _install_ntff_hook():
    """Shim antenv.axon_hooks (absent in this image) so trace=True works."""
    import sys
    import types

    try:
        import antenv.axon_hooks  # noqa: F401

        return
    except ImportError:
        pass
    try:
        import antenv
        from trn_agent_boot.trn_boot import _ntff_profile_via_ctypes

        hook = _ntff_profile_via_ctypes("/opt/axon/libaxon_pjrt.so")
        mod = types.ModuleType("antenv.axon_hooks")
        mod._hook = hook
        mod.get_axon_ntff_profile_hook = lambda: mod._hook
        mod.set_axon_ntff_profile_hook = lambda h: setattr(mod, "_hook", h)
        sys.modules["antenv.axon_hooks"] = mod
        antenv.axon_hooks = mod
    except Exception as e:  # degrade to no-trace
        print("ntff hook shim failed:", e)


def kernel(x, Wq, Wk, Wv, Wo, bo, _trace=False):
    global LAST_EXEC_TIME_NS
    if _trace:
        _install_ntff_hook()
    bo = np.asarray(bo, dtype=np.float32)
    nc = _get_nc()
    in_maps = make_in_maps(x, Wq, Wk, Wv, Wo)
    res = run_bass_kernel_spmd(nc, in_maps, list(range(NCORES)), trace=_trace)
    LAST_EXEC_TIME_NS = res.exec_time_ns
    out = np.empty((B, S, DOUT), dtype=np.float32)
    for b in range(B):
        out[b] = res.results[2 * b]["out"] + res.results[2 * b + 1]["out"] + bo
    return out


# revision 23
# speedup vs baseline: 1.1926x; 1.1045x over previous
"""Multi-head causal attention (B=4, S=2048, D=1024, H=16) on 8 trn2 NeuronCores.

Sharding: data-parallel over batch (4) x tensor-parallel over heads (2 groups
of 8 heads).  Core c handles batch c//2, head-group c%2.  Each core computes
its 512-wide slice of Q/K/V, causal attention for its 8 heads, and a partial
out-projection (row-parallel Wo).  The host sums the two partials per batch
and adds the bias (the "all-reduce" of the row-parallel out_proj).

Kernel layout notes (per core):
 - x arrives pre-transposed (and pre-cast to bf16) from host as xt
   [1024, 2048] so the contraction dim (d_in) is on partitions for all
   projection matmuls.
 - Q^T, K^T stored [d'=128 (2 heads), s] in bf16: directly usable as
   scores-matmul operands (S^T[k,q] = K^T_tile.T @ Q^T) with d on partitions.
 - V stored naturally [s, d'] with a ones-column appended per head (65-wide
   head slots) so the ctx matmul also produces the softmax denominators.
 - Scores are computed transposed (S^T: k on partitions, q free).  Softmax
   needs no max-stabilization (scores ~ N(0,1) after the 1/8 scale).  Causal
   masking: the fully-masked strip of a diagonal tile is memset to 0, the
   128x128 boundary block is multiplied by a precomputed triangular bf16
   mask, and only the live strip is exp'd.
 - Normalization is kept OFF the PE critical path: per head, the
   unnormalized ctx^T and the denominator row are copied out of PSUM by DVE
   (freeing the PSUM accumulator early); the reciprocal (DVE approx-fast,
   SBUF source only -- PSUM source is broken on HW), the K=1 broadcast
   matmuls, and the final normalize-multiplies are all batched at the end
   of the q-block where they pipeline behind other work.
 - The PE instruction queue is in-order, so phase overlap must be done at
   emission time: the projection matmuls for q-block n+1 are emitted as
   small "filler" quanta interleaved between the attention batches of
   q-block n.  While attention waits on ScalarE exp, PE executes projection
   quanta instead of idling (also keeping the HAM clock-gate warm).
"""

import numpy as np

import concourse.bacc as bacc
import concourse.mybir as mybir
from concourse import tile
from concourse.bass_utils import run_bass_kernel_spmd

F32 = mybir.dt.float32
BF16 = mybir.dt.bfloat16
EXP = mybir.ActivationFunctionType.Exp

B, S, DIN, DOUT, H = 4, 2048, 1024, 1024, 16
NCORES = 8
DG = 512          # d_out slice per core (8 heads)
NH = 8            # heads per core
HD = 64
NKT = DIN // 128  # 8 contraction tiles for projections
NQB = S // 512    # 4 q blocks of 512
NKB = S // 128    # 16 k blocks of 128
NDB = DG // 128   # 4 d'-blocks of 128 (2 heads each)

NP_BF16 = mybir.dt.np(BF16)

LAST_EXEC_TIME_NS = None


def build_nc():
    nc = bacc.Bacc()
    xt = nc.dram_tensor("xt", [DIN, S], BF16, kind="ExternalInput")
    wq = nc.dram_tensor("wq", [DIN, DG], BF16, kind="ExternalInput")
    wk = nc.dram_tensor("wk", [DIN, DG], BF16, kind="ExternalInput")
    wv = nc.dram_tensor("wv", [DIN, DG], BF16, kind="ExternalInput")
    wo = nc.dram_tensor("wo", [DG, DOUT], BF16, kind="ExternalInput")
    out = nc.dram_tensor("out", [S, DOUT], F32, kind="ExternalOutput")

    with tile.TileContext(nc) as tc:
        with (
            tc.tile_pool(name="persist", bufs=1) as persist,
            tc.tile_pool(name="xt", bufs=2) as xt_pool,
            tc.tile_pool(name="eb", bufs=4) as e_pool,
            tc.tile_pool(name="rp", bufs=2) as r_pool,
            tc.tile_pool(name="cu", bufs=9) as cu_pool,
            tc.tile_pool(name="ob", bufs=3) as o_pool,
            tc.tile_pool(name="psA", bufs=3, space="PSUM") as psA,
            tc.tile_pool(name="psC", bufs=2, space="PSUM") as psC,
        ):
            # ---- persistent SBUF tensors ----
            wq_sb = persist.tile([128, NKT, DG], BF16)
            wk_sb = persist.tile([128, NKT, DG], BF16)
            wv_sb = persist.tile([128, NKT, DG], BF16)
            wo_sb = persist.tile([128, NDB, DOUT], BF16)
            qt_sb = persist.tile([128, NDB, S], BF16)
            kt_sb = persist.tile([128, NDB, S], BF16)
            v_sb = persist.tile([128, NKB, NH, HD + 1], BF16)
            ct_sb = persist.tile([128, NDB, S], BF16)
            mask_sb = persist.tile([128, 128], BF16)
            ones_sb = persist.tile([1, 64], BF16)

            # ---- one-time setup ----
            nc.vector.memset(ones_sb[:], 1.0)
            nc.vector.memset(v_sb[:, :, :, HD : HD + 1], 1.0)
            nc.vector.memset(mask_sb[:], 1.0)
            # triangular causal boundary block: keep where q_local >= k_local
            nc.gpsimd.affine_select(
                out=mask_sb[:],
                in_=mask_sb[:],
                pattern=[[1, 128]],
                base=0,
                channel_multiplier=-1,
                compare_op=mybir.AluOpType.is_ge,
                fill=0.0,
            )

            xt_r = xt.rearrange("(kt p) s -> p kt s", p=128)
            xt_tiles = [None] * NQB

            def load_xt(n):
                t = xt_pool.tile([128, NKT, 512], BF16, tag="xt")
                nc.sync.dma_start(out=t[:], in_=xt_r[:, :, n * 512 : (n + 1) * 512])
                xt_tiles[n] = t

            # first x block before the weights so phase A(0) starts ASAP
            load_xt(0)
            for w_dram, w_sb in ((wq, wq_sb), (wk, wk_sb), (wv, wv_sb)):
                w_r = w_dram.rearrange("(kt p) d -> p kt d", p=128)
                for kt in range(NKT):
                    nc.sync.dma_start(out=w_sb[:, kt, :], in_=w_r[:, kt, :])
            wo_r = wo.rearrange("(t p) e -> p t e", p=128)
            for p in range(NDB):
                nc.sync.dma_start(out=wo_sb[:, p, :], in_=wo_r[:, p, :])

            def phase_a_quanta(n):
                """Emit projections for s-block n as a list of small closures.

                Each quantum is ~2 matmuls (or one PSUM->SBUF copy) so it can
                be interleaved between attention batches as PE filler.
                """
                quanta = []
                xt_t = xt_tiles[n]
                state = {}

                def q_group(w_sb, dst, mp):
                    def alloc():
                        state[("ps", w_sb.name, mp)] = psA.tile(
                            [128, 1024], F32, tag="ps", name=f"psa_{n}_{w_sb.name}_{mp}"
                        )

                    quanta.append(alloc)
                    for m01 in range(2):
                        m = mp * 2 + m01
                        for kt0 in range(0, NKT, 2):

                            def mm2(m01=m01, m=m, kt0=kt0, w_sb=w_sb, mp=mp):
                                ps = state[("ps", w_sb.name, mp)]
                                for kt in (kt0, kt0 + 1):
                                    nc.tensor.matmul(
                                        ps[:, m01 * 512 : (m01 + 1) * 512],
                                        lhsT=w_sb[:, kt, m * 128 : (m + 1) * 128],
                                        rhs=xt_t[:, kt, :],
                                        start=(kt == 0),
                                        stop=(kt == NKT - 1),
                                    )

                            quanta.append(mm2)

                    def cp(w_sb=w_sb, dst=dst, mp=mp):
                        ps = state[("ps", w_sb.name, mp)]
                        nc.vector.tensor_copy(
                            dst[:, mp * 2 : mp * 2 + 2, n * 512 : (n + 1) * 512],
                            ps.rearrange("p (m s) -> p m s", m=2),
                        )

                    quanta.append(cp)

                for w_sb, dst in ((wq_sb, qt_sb), (wk_sb, kt_sb)):
                    for mp in range(2):
                        q_group(w_sb, dst, mp)

                def v_group(sp):
                    def alloc(sp=sp):
                        state[("psv", sp)] = psA.tile([128, 1024], F32, tag="ps", name=f"psv_{n}_{sp}")

                    quanta.append(alloc)
                    for s01 in range(2):
                        ss = sp * 2 + s01
                        for kt0 in range(0, NKT, 2):

                            def mm2(s01=s01, ss=ss, kt0=kt0, sp=sp):
                                ps = state[("psv", sp)]
                                for kt in (kt0, kt0 + 1):
                                    nc.tensor.matmul(
                                        ps[:, s01 * 512 : (s01 + 1) * 512],
                                        lhsT=xt_t[:, kt, ss * 128 : (ss + 1) * 128],
                                        rhs=wv_sb[:, kt, :],
                                        start=(kt == 0),
                                        stop=(kt == NKT - 1),
                                    )

                            quanta.append(mm2)

                    def cp(sp=sp):
                        ps = state[("psv", sp)]
                        gss = n * 4 + sp * 2
                        nc.vector.tensor_copy(
                            v_sb[:, gss : gss + 2, :, 0:HD],
                            ps.rearrange("p (u h e) -> p u h e", u=2, e=HD),
                        )

                    quanta.append(cp)

                for sp in range(2):
                    v_group(sp)
                return quanta

            def phase_b(j, filler):
                """Attention for q-block j (heads sequential, batches of two
                k-tiles).  `filler` quanta are drained between batches to keep
                PE busy during exp waits."""
                nkb = 4 * j + 4
                nbatches = NH * (nkb // 2)
                nq = len(filler)
                drained = 0
                bi = 0
                pending = []  # (h, cu, rc) awaiting broadcast+multiply

                def finish_norm(h, cu, rc):
                    dblk, poff = h // 2, (h % 2) * 64
                    pb = psA.tile([64, 512], F32, tag="ps", name=f"pb_{j}_{h}")
                    nc.tensor.matmul(
                        pb[:], lhsT=ones_sb[:], rhs=rc[:], start=True, stop=True
                    )
                    rb = r_pool.tile([64, 512], BF16, tag="rb", bufs=2)
                    nc.scalar.copy(rb[:], pb[:])
                    nc.vector.tensor_mul(
                        ct_sb[poff : poff + 64, dblk, j * 512 : (j + 1) * 512],
                        cu[:],
                        rb[:],
                    )

                for h in range(NH):
                    dblk, poff = h // 2, (h % 2) * 64
                    if len(pending) > 1:
                        # norm for head h-2 is ready by now; finishing it here
                        # keeps the broadcast matmul off the critical path
                        finish_norm(*pending.pop(0))
                    pc = psC.tile([65, 512], F32, tag="pc", name=f"pc_{j}_{h}")
                    for ib in range(nkb // 2):
                        ps = psA.tile(
                            [128, 1024], F32, tag="ps", name=f"ps_{j}_{h}_{ib}"
                        )
                        for t in range(2):
                            i = 2 * ib + t
                            nc.tensor.matmul(
                                ps[:, t * 512 : (t + 1) * 512],
                                lhsT=kt_sb[
                                    poff : poff + 64, dblk, i * 128 : (i + 1) * 128
                                ],
                                rhs=qt_sb[
                                    poff : poff + 64, dblk, j * 512 : (j + 1) * 512
                                ],
                                start=True,
                                stop=True,
                            )
                        eb = e_pool.tile(
                            [128, 1024], BF16, tag="eb", name=f"eb_{j}_{h}_{ib}"
                        )
                        d1 = 2 * ib + 1 - 4 * j
                        if d1 < 0:
                            nc.scalar.activation(eb[:], ps[:], EXP, scale=0.125)
                        else:
                            for t in range(2):
                                i = 2 * ib + t
                                dd = i - 4 * j
                                lo = t * 512
                                if dd < 0:
                                    nc.scalar.activation(
                                        eb[:, lo : lo + 512],
                                        ps[:, lo : lo + 512],
                                        EXP,
                                        scale=0.125,
                                    )
                                else:
                                    z = 128 * dd
                                    if z > 0:
                                        nc.vector.memset(eb[:, lo : lo + z], 0.0)
                                    nc.scalar.activation(
                                        eb[:, lo + z : lo + 512],
                                        ps[:, lo + z : lo + 512],
                                        EXP,
                                        scale=0.125,
                                    )
                                    nc.vector.tensor_mul(
                                        eb[:, lo + z : lo + z + 128],
                                        eb[:, lo + z : lo + z + 128],
                                        mask_sb[:],
                                    )
                        for t in range(2):
                            i = 2 * ib + t
                            nc.tensor.matmul(
                                pc[:],
                                lhsT=v_sb[:, i, h, :],
                                rhs=eb[:, t * 512 : (t + 1) * 512],
                                start=(i == 0),
                                stop=(i == nkb - 1),
                            )
                        bi += 1
                        want = nq * bi // nbatches
                        while drained < want:
                            filler[drained]()
                            drained += 1
                    # pull ctx + denominator out of PSUM (DVE only), free pc
                    dn = r_pool.tile([1, 512], F32, tag="dn", bufs=3)
                    nc.vector.tensor_copy(dn[:], pc[64:65, :])
                    rc32 = r_pool.tile([1, 512], F32, tag="rc32", bufs=3)
                    nc.vector.reciprocal_approx_fast(rc32[:], dn[:])
                    rc = r_pool.tile([1, 512], BF16, tag="rc", bufs=4)
                    nc.vector.tensor_copy(rc[:], rc32[:])
                    cu = cu_pool.tile([64, 512], BF16, tag="cu")
                    nc.vector.tensor_copy(cu[:], pc[0:64, :])
                    pending.append((h, cu, rc))
                while drained < nq:
                    filler[drained]()
                    drained += 1
                for ent in pending:
                    finish_norm(*ent)

            def phase_c_quanta(n):
                quanta = []
                for qq in range(4 * n, 4 * n + 4):
                    for e2 in range(2):

                        def unit(qq=qq, e2=e2):
                            po = psA.tile(
                                [128, 512], F32, tag="ps", name=f"po_{qq}_{e2}"
                            )
                            for p in range(NDB):
                                nc.tensor.matmul(
                                    po[:],
                                    lhsT=ct_sb[:, p, qq * 128 : (qq + 1) * 128],
                                    rhs=wo_sb[:, p, e2 * 512 : (e2 + 1) * 512],
                                    start=(p == 0),
                                    stop=(p == NDB - 1),
                                )
                            ob = o_pool.tile(
                                [128, 512], F32, tag="ob", name=f"ob_{qq}_{e2}"
                            )
                            nc.vector.tensor_copy(ob[:], po[:])
                            nc.sync.dma_start(
                                out=out[
                                    qq * 128 : (qq + 1) * 128,
                                    e2 * 512 : (e2 + 1) * 512,
                                ],
                                in_=ob[:],
                            )

                        quanta.append(unit)
                return quanta

            # ---- main schedule ----
            # A(0) runs plain; B(n) is interleaved with A(n+1) + C(n-1).
            for q in phase_a_quanta(0):
                q()
            for n in range(NQB):
                filler = []
                if n >= 1:
                    filler += phase_c_quanta(n - 1)
                if n + 1 < NQB:
                    load_xt(n + 1)
                    filler += phase_a_quanta(n + 1)
                phase_b(n, filler)
            for q in phase_c_quanta(NQB - 1):
                q()
    nc.compile()
    return nc


_NC_CACHE = None


def _get_nc():
    global _NC_CACHE
    if _NC_CACHE is None:
        _NC_CACHE = build_nc()
    return _NC_CACHE


def make_in_maps(x, Wq, Wk, Wv, Wo):
    x = np.asarray(x, dtype=np.float32).astype(NP_BF16)
    Wq = np.asarray(Wq, dtype=np.float32).astype(NP_BF16)
    Wk = np.asarray(Wk, dtype=np.float32).astype(NP_BF16)
    Wv = np.asarray(Wv, dtype=np.float32).astype(NP_BF16)
    Wo = np.asarray(Wo, dtype=np.float32).astype(NP_BF16)
    in_maps = []
    for c in range(NCORES):
        b, g = c // 2, c % 2
        sl = slice(g * DG, (g + 1) * DG)
        in_maps.append(
            {
                "xt": np.ascontiguousarray(x[b].T),
                "wq": np.ascontiguousarray(Wq[:, sl]),
                "wk": np.ascontiguousarray(Wk[:, sl]),
                "wv": np.ascontiguousarray(Wv[:, sl]),
                "wo": np.ascontiguousarray(Wo[sl, :]),
            }
        )
    return in_maps


def _install_ntff_hook():
    """Shim antenv.axon_hooks (absent in this image) so trace=True works."""
    import sys
    import types

    try:
        import antenv.axon_hooks  # noqa: F401

        return
    except ImportError:
        pass
    try:
        import antenv
        from trn_agent_boot.trn_boot import _ntff_profile_via_ctypes

        hook = _ntff_profile_via_ctypes("/opt/axon/libaxon_pjrt.so")
        mod = types.ModuleType("antenv.axon_hooks")
        mod._hook = hook
        mod.get_axon_ntff_profile_hook = lambda: mod._hook
        mod.set_axon_ntff_profile_hook = lambda h: setattr(mod, "_hook", h)
        sys.modules["antenv.axon_hooks"] = mod
        antenv.axon_hooks = mod
    except Exception as e:  # degrade to no-trace
        print("ntff hook shim failed:", e)


def kernel(x, Wq, Wk, Wv, Wo, bo, _trace=False):
    global LAST_EXEC_TIME_NS
    if _trace:
        _install_ntff_hook()
    bo = np.asarray(bo, dtype=np.float32)
    nc = _get_nc()
    in_maps = make_in_maps(x, Wq, Wk, Wv, Wo)
    res = run_bass_kernel_spmd(nc, in_maps, list(range(NCORES)), trace=_trace)
    LAST_EXEC_TIME_NS = res.exec_time_ns
    out = np.empty((B, S, DOUT), dtype=np.float32)
    for b in range(B):
        out[b] = res.results[2 * b]["out"] + res.results[2 * b + 1]["out"] + bo
    return out


# revision 24
# speedup vs baseline: 1.2214x; 1.0241x over previous
"""Multi-head causal attention (B=4, S=2048, D=1024, H=16) on 8 trn2 NeuronCores.

Sharding: data-parallel over batch (4) x tensor-parallel over heads (2 groups
of 8 heads).  Core c handles batch c//2, head-group c%2.  Each core computes
its 512-wide slice of Q/K/V, causal attention for its 8 heads, and a partial
out-projection (row-parallel Wo).  The host sums the two partials per batch
and adds the bias (the "all-reduce" of the row-parallel out_proj).

Kernel layout notes (per core):
 - x arrives pre-transposed (and pre-cast to bf16) from host as xt
   [1024, 2048] so the contraction dim (d_in) is on partitions for all
   projection matmuls.
 - Q^T, K^T stored [d'=128 (2 heads), s] in bf16: directly usable as
   scores-matmul operands (S^T[k,q] = K^T_tile.T @ Q^T) with d on partitions.
 - V stored naturally [s, d'] with a ones-column appended per head (65-wide
   head slots) so the ctx matmul also produces the softmax denominators.
 - Scores are computed transposed (S^T: k on partitions, q free).  Softmax
   needs no max-stabilization (scores ~ N(0,1) after the 1/8 scale).  Causal
   masking: the fully-masked strip of a diagonal tile is memset to 0, the
   128x128 boundary block is multiplied by a precomputed triangular bf16
   mask, and only the live strip is exp'd.
 - Normalization is kept OFF the PE critical path: per head, the
   unnormalized ctx^T and the denominator row are copied out of PSUM by DVE
   (freeing the PSUM accumulator early); the reciprocal (DVE approx-fast,
   SBUF source only -- PSUM source is broken on HW), the K=1 broadcast
   matmuls, and the final normalize-multiplies are all batched at the end
   of the q-block where they pipeline behind other work.
 - The PE instruction queue is in-order, so phase overlap must be done at
   emission time: the projection matmuls for q-block n+1 are emitted as
   small "filler" quanta interleaved between the attention batches of
   q-block n.  While attention waits on ScalarE exp, PE executes projection
   quanta instead of idling (also keeping the HAM clock-gate warm).
"""

import numpy as np

import concourse.bacc as bacc
import concourse.mybir as mybir
from concourse import tile
from concourse.bass_utils import run_bass_kernel_spmd

F32 = mybir.dt.float32
BF16 = mybir.dt.bfloat16
EXP = mybir.ActivationFunctionType.Exp

B, S, DIN, DOUT, H = 4, 2048, 1024, 1024, 16
NCORES = 8
DG = 512          # d_out slice per core (8 heads)
NH = 8            # heads per core
HD = 64
NKT = DIN // 128  # 8 contraction tiles for projections
NQB = S // 512    # 4 q blocks of 512
NKB = S // 128    # 16 k blocks of 128
NDB = DG // 128   # 4 d'-blocks of 128 (2 heads each)

NP_BF16 = mybir.dt.np(BF16)

LAST_EXEC_TIME_NS = None


def build_nc():
    nc = bacc.Bacc()
    xt = nc.dram_tensor("xt", [DIN, S], BF16, kind="ExternalInput")
    wq = nc.dram_tensor("wq", [DIN, DG], BF16, kind="ExternalInput")
    wk = nc.dram_tensor("wk", [DIN, DG], BF16, kind="ExternalInput")
    wv = nc.dram_tensor("wv", [DIN, DG], BF16, kind="ExternalInput")
    wo = nc.dram_tensor("wo", [DG, DOUT], BF16, kind="ExternalInput")
    out = nc.dram_tensor("out", [S, DOUT], F32, kind="ExternalOutput")

    with tile.TileContext(nc) as tc:
        with (
            tc.tile_pool(name="persist", bufs=1) as persist,
            tc.tile_pool(name="xt", bufs=2) as xt_pool,
            tc.tile_pool(name="eb", bufs=4) as e_pool,
            tc.tile_pool(name="rp", bufs=2) as r_pool,
            tc.tile_pool(name="cu", bufs=9) as cu_pool,
            tc.tile_pool(name="ob", bufs=3) as o_pool,
            tc.tile_pool(name="psA", bufs=3, space="PSUM") as psA,
            tc.tile_pool(name="psC", bufs=2, space="PSUM") as psC,
        ):
            # ---- persistent SBUF tensors ----
            wq_sb = persist.tile([128, NKT, DG], BF16)
            wk_sb = persist.tile([128, NKT, DG], BF16)
            wv_sb = persist.tile([128, NKT, DG], BF16)
            wo_sb = persist.tile([128, NDB, DOUT], BF16)
            qt_sb = persist.tile([128, NDB, S], BF16)
            kt_sb = persist.tile([128, NDB, S], BF16)
            v_sb = persist.tile([128, NKB, NH, HD + 1], BF16)
            ct_sb = persist.tile([128, NDB, S], BF16)
            mask_sb = persist.tile([128, 128], BF16)
            ones_sb = persist.tile([1, 64], BF16)

            # ---- one-time setup ----
            nc.vector.memset(ones_sb[:], 1.0)
            nc.vector.memset(v_sb[:, :, :, HD : HD + 1], 1.0)
            nc.vector.memset(mask_sb[:], 1.0)
            # triangular causal boundary block: keep where q_local >= k_local
            nc.gpsimd.affine_select(
                out=mask_sb[:],
                in_=mask_sb[:],
                pattern=[[1, 128]],
                base=0,
                channel_multiplier=-1,
                compare_op=mybir.AluOpType.is_ge,
                fill=0.0,
            )

            xt_r = xt.rearrange("(kt p) s -> p kt s", p=128)
            xt_tiles = [None] * NQB

            def load_xt(n):
                t = xt_pool.tile([128, NKT, 512], BF16, tag="xt")
                for kt in range(NKT):
                    nc.sync.dma_start(
                        out=t[:, kt, :],
                        in_=xt_r[:, kt, n * 512 : (n + 1) * 512],
                    )
                xt_tiles[n] = t

            # first x block before the weights so phase A(0) starts ASAP
            load_xt(0)
            for w_dram, w_sb in ((wq, wq_sb), (wk, wk_sb), (wv, wv_sb)):
                w_r = w_dram.rearrange("(kt p) d -> p kt d", p=128)
                for kt in range(NKT):
                    nc.sync.dma_start(out=w_sb[:, kt, :], in_=w_r[:, kt, :])
            wo_r = wo.rearrange("(t p) e -> p t e", p=128)
            for p in range(NDB):
                nc.sync.dma_start(out=wo_sb[:, p, :], in_=wo_r[:, p, :])

            def phase_a_quanta(n):
                """Emit projections for s-block n as a list of small closures.

                Each quantum is ~2 matmuls (or one PSUM->SBUF copy) so it can
                be interleaved between attention batches as PE filler.
                """
                quanta = []
                xt_t = xt_tiles[n]
                state = {}

                def q_group(w_sb, dst, mp):
                    def alloc():
                        state[("ps", w_sb.name, mp)] = psA.tile(
                            [128, 1024], F32, tag="ps", name=f"psa_{n}_{w_sb.name}_{mp}"
                        )

                    quanta.append(alloc)
                    for m01 in range(2):
                        m = mp * 2 + m01
                        for kt0 in range(0, NKT, 2):

                            def mm2(m01=m01, m=m, kt0=kt0, w_sb=w_sb, mp=mp):
                                ps = state[("ps", w_sb.name, mp)]
                                for kt in (kt0, kt0 + 1):
                                    nc.tensor.matmul(
                                        ps[:, m01 * 512 : (m01 + 1) * 512],
                                        lhsT=w_sb[:, kt, m * 128 : (m + 1) * 128],
                                        rhs=xt_t[:, kt, :],
                                        start=(kt == 0),
                                        stop=(kt == NKT - 1),
                                    )

                            quanta.append(mm2)

                    def cp(w_sb=w_sb, dst=dst, mp=mp):
                        ps = state[("ps", w_sb.name, mp)]
                        nc.vector.tensor_copy(
                            dst[:, mp * 2 : mp * 2 + 2, n * 512 : (n + 1) * 512],
                            ps.rearrange("p (m s) -> p m s", m=2),
                        )

                    quanta.append(cp)

                for w_sb, dst in ((wq_sb, qt_sb), (wk_sb, kt_sb)):
                    for mp in range(2):
                        q_group(w_sb, dst, mp)

                def v_group(sp):
                    def alloc(sp=sp):
                        state[("psv", sp)] = psA.tile([128, 1024], F32, tag="ps", name=f"psv_{n}_{sp}")

                    quanta.append(alloc)
                    for s01 in range(2):
                        ss = sp * 2 + s01
                        for kt0 in range(0, NKT, 2):

                            def mm2(s01=s01, ss=ss, kt0=kt0, sp=sp):
                                ps = state[("psv", sp)]
                                for kt in (kt0, kt0 + 1):
                                    nc.tensor.matmul(
                                        ps[:, s01 * 512 : (s01 + 1) * 512],
                                        lhsT=xt_t[:, kt, ss * 128 : (ss + 1) * 128],
                                        rhs=wv_sb[:, kt, :],
                                        start=(kt == 0),
                                        stop=(kt == NKT - 1),
                                    )

                            quanta.append(mm2)

                    def cp(sp=sp):
                        ps = state[("psv", sp)]
                        gss = n * 4 + sp * 2
                        nc.vector.tensor_copy(
                            v_sb[:, gss : gss + 2, :, 0:HD],
                            ps.rearrange("p (u h e) -> p u h e", u=2, e=HD),
                        )

                    quanta.append(cp)

                for sp in range(2):
                    v_group(sp)
                return quanta

            def phase_b(j, filler):
                """Attention for q-block j (heads sequential, batches of two
                k-tiles).  `filler` quanta are drained between batches to keep
                PE busy during exp waits."""
                nkb = 4 * j + 4
                nbatches = NH * (nkb // 2)
                nq = len(filler)
                drained = 0
                bi = 0
                pending = []  # (h, cu, rc) awaiting broadcast+multiply

                def finish_norm(h, cu, rc):
                    dblk, poff = h // 2, (h % 2) * 64
                    pb = psA.tile([64, 512], F32, tag="ps", name=f"pb_{j}_{h}")
                    nc.tensor.matmul(
                        pb[:], lhsT=ones_sb[:], rhs=rc[:], start=True, stop=True
                    )
                    rb = r_pool.tile([64, 512], BF16, tag="rb", bufs=2)
                    nc.scalar.copy(rb[:], pb[:])
                    nc.vector.tensor_mul(
                        ct_sb[poff : poff + 64, dblk, j * 512 : (j + 1) * 512],
                        cu[:],
                        rb[:],
                    )

                for h in range(NH):
                    dblk, poff = h // 2, (h % 2) * 64
                    if len(pending) > 1:
                        # norm for head h-2 is ready by now; finishing it here
                        # keeps the broadcast matmul off the critical path
                        finish_norm(*pending.pop(0))
                    pc = psC.tile([65, 512], F32, tag="pc", name=f"pc_{j}_{h}")
                    for ib in range(nkb // 2):
                        ps = psA.tile(
                            [128, 1024], F32, tag="ps", name=f"ps_{j}_{h}_{ib}"
                        )
                        for t in range(2):
                            i = 2 * ib + t
                            nc.tensor.matmul(
                                ps[:, t * 512 : (t + 1) * 512],
                                lhsT=kt_sb[
                                    poff : poff + 64, dblk, i * 128 : (i + 1) * 128
                                ],
                                rhs=qt_sb[
                                    poff : poff + 64, dblk, j * 512 : (j + 1) * 512
                                ],
                                start=True,
                                stop=True,
                            )
                        eb = e_pool.tile(
                            [128, 1024], BF16, tag="eb", name=f"eb_{j}_{h}_{ib}"
                        )
                        d1 = 2 * ib + 1 - 4 * j
                        if d1 < 0:
                            nc.scalar.activation(eb[:], ps[:], EXP, scale=0.125)
                        else:
                            for t in range(2):
                                i = 2 * ib + t
                                dd = i - 4 * j
                                lo = t * 512
                                if dd < 0:
                                    nc.scalar.activation(
                                        eb[:, lo : lo + 512],
                                        ps[:, lo : lo + 512],
                                        EXP,
                                        scale=0.125,
                                    )
                                else:
                                    z = 128 * dd
                                    if z > 0:
                                        nc.vector.memset(eb[:, lo : lo + z], 0.0)
                                    nc.scalar.activation(
                                        eb[:, lo + z : lo + 512],
                                        ps[:, lo + z : lo + 512],
                                        EXP,
                                        scale=0.125,
                                    )
                                    nc.vector.tensor_mul(
                                        eb[:, lo + z : lo + z + 128],
                                        eb[:, lo + z : lo + z + 128],
                                        mask_sb[:],
                                    )
                        for t in range(2):
                            i = 2 * ib + t
                            nc.tensor.matmul(
                                pc[:],
                                lhsT=v_sb[:, i, h, :],
                                rhs=eb[:, t * 512 : (t + 1) * 512],
                                start=(i == 0),
                                stop=(i == nkb - 1),
                            )
                        bi += 1
                        want = nq * bi // nbatches
                        while drained < want:
                            filler[drained]()
                            drained += 1
                    # pull ctx + denominator out of PSUM (DVE only), free pc
                    dn = r_pool.tile([1, 512], F32, tag="dn", bufs=3)
                    nc.vector.tensor_copy(dn[:], pc[64:65, :])
                    rc32 = r_pool.tile([1, 512], F32, tag="rc32", bufs=3)
                    nc.vector.reciprocal_approx_fast(rc32[:], dn[:])
                    rc = r_pool.tile([1, 512], BF16, tag="rc", bufs=4)
                    nc.vector.tensor_copy(rc[:], rc32[:])
                    cu = cu_pool.tile([64, 512], BF16, tag="cu")
                    nc.vector.tensor_copy(cu[:], pc[0:64, :])
                    pending.append((h, cu, rc))
                while drained < nq:
                    filler[drained]()
                    drained += 1
                return [
                    (lambda ent=ent: finish_norm(*ent)) for ent in pending
                ]

            def phase_c_quanta(n):
                quanta = []
                for qq in range(4 * n, 4 * n + 4):
                    for e2 in range(2):

                        def unit(qq=qq, e2=e2):
                            po = psA.tile(
                                [128, 512], F32, tag="ps", name=f"po_{qq}_{e2}"
                            )
                            for p in range(NDB):
                                nc.tensor.matmul(
                                    po[:],
                                    lhsT=ct_sb[:, p, qq * 128 : (qq + 1) * 128],
                                    rhs=wo_sb[:, p, e2 * 512 : (e2 + 1) * 512],
                                    start=(p == 0),
                                    stop=(p == NDB - 1),
                                )
                            ob = o_pool.tile(
                                [128, 512], F32, tag="ob", name=f"ob_{qq}_{e2}"
                            )
                            nc.vector.tensor_copy(ob[:], po[:])
                            nc.sync.dma_start(
                                out=out[
                                    qq * 128 : (qq + 1) * 128,
                                    e2 * 512 : (e2 + 1) * 512,
                                ],
                                in_=ob[:],
                            )

                        quanta.append(unit)
                return quanta

            # ---- main schedule ----
            # A(0) runs plain; B(n) is interleaved with the previous block's
            # leftover normalizations + C(n-1) + A(n+1).
            for q in phase_a_quanta(0):
                q()
            leftover = []
            for n in range(NQB):
                filler = list(leftover)
                if n >= 1:
                    filler += phase_c_quanta(n - 1)
                if n + 1 < NQB:
                    load_xt(n + 1)
                    filler += phase_a_quanta(n + 1)
                leftover = phase_b(n, filler)
            for q in leftover:
                q()
            for q in phase_c_quanta(NQB - 1):
                q()
    nc.compile()
    return nc


_NC_CACHE = None


def _get_nc():
    global _NC_CACHE
    if _NC_CACHE is None:
        _NC_CACHE = build_nc()
    return _NC_CACHE


def make_in_maps(x, Wq, Wk, Wv, Wo):
    x = np.asarray(x, dtype=np.float32).astype(NP_BF16)
    Wq = np.asarray(Wq, dtype=np.float32).astype(NP_BF16)
    Wk = np.asarray(Wk, dtype=np.float32).astype(NP_BF16)
    Wv = np.asarray(Wv, dtype=np.float32).astype(NP_BF16)
    Wo = np.asarray(Wo, dtype=np.float32).astype(NP_BF16)
    in_maps = []
    for c in range(NCORES):
        b, g = c // 2, c % 2
        sl = slice(g * DG, (g + 1) * DG)
        in_maps.append(
            {
                "xt": np.ascontiguousarray(x[b].T),
                "wq": np.ascontiguousarray(Wq[:, sl]),
                "wk": np.ascontiguousarray(Wk[:, sl]),
                "wv": np.ascontiguousarray(Wv[:, sl]),
                "wo": np.ascontiguousarray(Wo[sl, :]),
            }
        )
    return in_maps


def _install_ntff_hook():
    """Shim antenv.axon_hooks (absent in this image) so trace=True works."""
    import sys
    import types

    try:
        import antenv.axon_hooks  # noqa: F401

        return
    except ImportError:
        pass
    try:
        import antenv
        from trn_agent_boot.trn_boot import _ntff_profile_via_ctypes

        hook = _ntff_profile_via_ctypes("/opt/axon/libaxon_pjrt.so")
        mod = types.ModuleType("antenv.axon_hooks")
        mod._hook = hook
        mod.get_axon_ntff_profile_hook = lambda: mod._hook
        mod.set_axon_ntff_profile_hook = lambda h: setattr(mod, "_hook", h)
        sys.modules["antenv.axon_hooks"] = mod
        antenv.axon_hooks = mod
    except Exception as e:  # degrade to no-trace
        print("ntff hook shim failed:", e)


def kernel(x, Wq, Wk, Wv, Wo, bo, _trace=False):
    global LAST_EXEC_TIME_NS
    if _trace:
        _install_ntff_hook()
    bo = np.asarray(bo, dtype=np.float32)
    nc = _get_nc()
    in_maps = make_in_maps(x, Wq, Wk, Wv, Wo)
    res = run_bass_kernel_spmd(nc, in_maps, list(range(NCORES)), trace=_trace)
    LAST_EXEC_TIME_NS = res.exec_time_ns
    out = np.empty((B, S, DOUT), dtype=np.float32)
    for b in range(B):
        out[b] = res.results[2 * b]["out"] + res.results[2 * b + 1]["out"] + bo
    return out


# revision 25
# speedup vs baseline: 1.2276x; 1.0051x over previous
"""Multi-head causal attention (B=4, S=2048, D=1024, H=16) on 8 trn2 NeuronCores.

Sharding: data-parallel over batch (4) x tensor-parallel over heads (2 groups
of 8 heads).  Core c handles batch c//2, head-group c%2.  Each core computes
its 512-wide slice of Q/K/V, causal attention for its 8 heads, and a partial
out-projection (row-parallel Wo).  The host sums the two partials per batch
and adds the bias (the "all-reduce" of the row-parallel out_proj).

Kernel layout notes (per core):
 - x arrives pre-transposed (and pre-cast to bf16) from host as xt
   [1024, 2048] so the contraction dim (d_in) is on partitions for all
   projection matmuls.
 - Q^T, K^T stored [d'=128 (2 heads), s] in bf16: directly usable as
   scores-matmul operands (S^T[k,q] = K^T_tile.T @ Q^T) with d on partitions.
 - V stored naturally [s, d'] with a ones-column appended per head (65-wide
   head slots) so the ctx matmul also produces the softmax denominators.
 - Scores are computed transposed (S^T: k on partitions, q free).  Softmax
   needs no max-stabilization (scores ~ N(0,1) after the 1/8 scale).  Causal
   masking: the fully-masked strip of a diagonal tile is memset to 0, the
   128x128 boundary block is multiplied by a precomputed triangular bf16
   mask, and only the live strip is exp'd.
 - Normalization is kept OFF the PE critical path: per head, the
   unnormalized ctx^T and the denominator row are copied out of PSUM by DVE
   (freeing the PSUM accumulator early); the reciprocal (DVE approx-fast,
   SBUF source only -- PSUM source is broken on HW), the K=1 broadcast
   matmuls, and the final normalize-multiplies are all batched at the end
   of the q-block where they pipeline behind other work.
 - The PE instruction queue is in-order, so phase overlap must be done at
   emission time: the projection matmuls for q-block n+1 are emitted as
   small "filler" quanta interleaved between the attention batches of
   q-block n.  While attention waits on ScalarE exp, PE executes projection
   quanta instead of idling (also keeping the HAM clock-gate warm).
"""

import numpy as np

import concourse.bacc as bacc
import concourse.mybir as mybir
from concourse import tile
from concourse.bass_utils import run_bass_kernel_spmd

F32 = mybir.dt.float32
BF16 = mybir.dt.bfloat16
EXP = mybir.ActivationFunctionType.Exp

B, S, DIN, DOUT, H = 4, 2048, 1024, 1024, 16
NCORES = 8
DG = 512          # d_out slice per core (8 heads)
NH = 8            # heads per core
HD = 64
NKT = DIN // 128  # 8 contraction tiles for projections
NQB = S // 512    # 4 q blocks of 512
NKB = S // 128    # 16 k blocks of 128
NDB = DG // 128   # 4 d'-blocks of 128 (2 heads each)

NP_BF16 = mybir.dt.np(BF16)

LAST_EXEC_TIME_NS = None


def build_nc():
    nc = bacc.Bacc()
    xt = nc.dram_tensor("xt", [DIN, S], BF16, kind="ExternalInput")
    wq = nc.dram_tensor("wq", [DIN, DG], BF16, kind="ExternalInput")
    wk = nc.dram_tensor("wk", [DIN, DG], BF16, kind="ExternalInput")
    wv = nc.dram_tensor("wv", [DIN, DG], BF16, kind="ExternalInput")
    wo = nc.dram_tensor("wo", [DG, DOUT], BF16, kind="ExternalInput")
    out = nc.dram_tensor("out", [S, DOUT], F32, kind="ExternalOutput")

    with tile.TileContext(nc) as tc:
        with (
            tc.tile_pool(name="persist", bufs=1) as persist,
            tc.tile_pool(name="xt", bufs=2) as xt_pool,
            tc.tile_pool(name="eb", bufs=4) as e_pool,
            tc.tile_pool(name="rp", bufs=2) as r_pool,
            tc.tile_pool(name="cu", bufs=9) as cu_pool,
            tc.tile_pool(name="ob", bufs=3) as o_pool,
            tc.tile_pool(name="psA", bufs=3, space="PSUM") as psA,
            tc.tile_pool(name="psC", bufs=2, space="PSUM") as psC,
        ):
            # ---- persistent SBUF tensors ----
            wq_sb = persist.tile([128, NKT, DG], BF16)
            wk_sb = persist.tile([128, NKT, DG], BF16)
            wv_sb = persist.tile([128, NKT, DG], BF16)
            wo_sb = persist.tile([128, NDB, DOUT], BF16)
            qt_sb = persist.tile([128, NDB, S], BF16)
            kt_sb = persist.tile([128, NDB, S], BF16)
            v_sb = persist.tile([128, NKB, NH, HD + 1], BF16)
            ct_sb = persist.tile([128, NDB, S], BF16)
            mask_sb = persist.tile([128, 128], BF16)
            ones_sb = persist.tile([1, 64], BF16)

            # ---- one-time setup ----
            nc.vector.memset(ones_sb[:], 1.0)
            nc.vector.memset(v_sb[:, :, :, HD : HD + 1], 1.0)
            nc.vector.memset(mask_sb[:], 1.0)
            # triangular causal boundary block: keep where q_local >= k_local
            nc.gpsimd.affine_select(
                out=mask_sb[:],
                in_=mask_sb[:],
                pattern=[[1, 128]],
                base=0,
                channel_multiplier=-1,
                compare_op=mybir.AluOpType.is_ge,
                fill=0.0,
            )

            xt_r = xt.rearrange("(kt p) s -> p kt s", p=128)
            xt_tiles = [None] * NQB

            def load_xt(n):
                t = xt_pool.tile([128, NKT, 512], BF16, tag="xt")
                for kt in range(NKT):
                    nc.sync.dma_start(
                        out=t[:, kt, :],
                        in_=xt_r[:, kt, n * 512 : (n + 1) * 512],
                    )
                xt_tiles[n] = t

            # first x block before the weights so phase A(0) starts ASAP
            load_xt(0)
            for w_dram, w_sb in ((wq, wq_sb), (wk, wk_sb), (wv, wv_sb)):
                w_r = w_dram.rearrange("(kt p) d -> p kt d", p=128)
                for kt in range(NKT):
                    nc.sync.dma_start(out=w_sb[:, kt, :], in_=w_r[:, kt, :])
            wo_r = wo.rearrange("(t p) e -> p t e", p=128)
            for p in range(NDB):
                nc.sync.dma_start(out=wo_sb[:, p, :], in_=wo_r[:, p, :])

            def phase_a_quanta(n):
                """Emit projections for s-block n as a list of small closures.

                Each quantum is ~2 matmuls (or one PSUM->SBUF copy) so it can
                be interleaved between attention batches as PE filler.
                """
                quanta = []
                xt_t = xt_tiles[n]
                state = {}

                def q_group(w_sb, dst, mp):
                    def alloc():
                        state[("ps", w_sb.name, mp)] = psA.tile(
                            [128, 1024], F32, tag="ps", name=f"psa_{n}_{w_sb.name}_{mp}"
                        )

                    quanta.append(alloc)
                    for kt in range(NKT):

                        def mm2(kt=kt, w_sb=w_sb, mp=mp):
                            ps = state[("ps", w_sb.name, mp)]
                            for m01 in range(2):  # alternate psum banks
                                m = mp * 2 + m01
                                nc.tensor.matmul(
                                    ps[:, m01 * 512 : (m01 + 1) * 512],
                                    lhsT=w_sb[:, kt, m * 128 : (m + 1) * 128],
                                    rhs=xt_t[:, kt, :],
                                    start=(kt == 0),
                                    stop=(kt == NKT - 1),
                                )

                        quanta.append(mm2)

                    def cp(w_sb=w_sb, dst=dst, mp=mp):
                        ps = state[("ps", w_sb.name, mp)]
                        nc.vector.tensor_copy(
                            dst[:, mp * 2 : mp * 2 + 2, n * 512 : (n + 1) * 512],
                            ps.rearrange("p (m s) -> p m s", m=2),
                        )

                    quanta.append(cp)

                for w_sb, dst in ((wq_sb, qt_sb), (wk_sb, kt_sb)):
                    for mp in range(2):
                        q_group(w_sb, dst, mp)

                def v_group(sp):
                    def alloc(sp=sp):
                        state[("psv", sp)] = psA.tile([128, 1024], F32, tag="ps", name=f"psv_{n}_{sp}")

                    quanta.append(alloc)
                    for kt in range(NKT):

                        def mm2(kt=kt, sp=sp):
                            ps = state[("psv", sp)]
                            for s01 in range(2):  # alternate psum banks
                                ss = sp * 2 + s01
                                nc.tensor.matmul(
                                    ps[:, s01 * 512 : (s01 + 1) * 512],
                                    lhsT=xt_t[:, kt, ss * 128 : (ss + 1) * 128],
                                    rhs=wv_sb[:, kt, :],
                                    start=(kt == 0),
                                    stop=(kt == NKT - 1),
                                )

                        quanta.append(mm2)

                    def cp(sp=sp):
                        ps = state[("psv", sp)]
                        gss = n * 4 + sp * 2
                        nc.vector.tensor_copy(
                            v_sb[:, gss : gss + 2, :, 0:HD],
                            ps.rearrange("p (u h e) -> p u h e", u=2, e=HD),
                        )

                    quanta.append(cp)

                for sp in range(2):
                    v_group(sp)
                return quanta

            def phase_b(j, filler):
                """Attention for q-block j (heads sequential, batches of two
                k-tiles).  `filler` quanta are drained between batches to keep
                PE busy during exp waits."""
                nkb = 4 * j + 4
                nbatches = NH * (nkb // 2)
                nq = len(filler)
                drained = 0
                bi = 0
                pending = []  # (h, cu, rc) awaiting broadcast+multiply

                def finish_norm(h, cu, rc):
                    dblk, poff = h // 2, (h % 2) * 64
                    pb = psA.tile([64, 512], F32, tag="ps", name=f"pb_{j}_{h}")
                    nc.tensor.matmul(
                        pb[:], lhsT=ones_sb[:], rhs=rc[:], start=True, stop=True
                    )
                    rb = r_pool.tile([64, 512], BF16, tag="rb", bufs=2)
                    nc.scalar.copy(rb[:], pb[:])
                    nc.vector.tensor_mul(
                        ct_sb[poff : poff + 64, dblk, j * 512 : (j + 1) * 512],
                        cu[:],
                        rb[:],
                    )

                for h in range(NH):
                    dblk, poff = h // 2, (h % 2) * 64
                    if len(pending) > 1:
                        # norm for head h-2 is ready by now; finishing it here
                        # keeps the broadcast matmul off the critical path
                        finish_norm(*pending.pop(0))
                    pc = psC.tile([65, 512], F32, tag="pc", name=f"pc_{j}_{h}")
                    for ib in range(nkb // 2):
                        ps = psA.tile(
                            [128, 1024], F32, tag="ps", name=f"ps_{j}_{h}_{ib}"
                        )
                        for t in range(2):
                            i = 2 * ib + t
                            nc.tensor.matmul(
                                ps[:, t * 512 : (t + 1) * 512],
                                lhsT=kt_sb[
                                    poff : poff + 64, dblk, i * 128 : (i + 1) * 128
                                ],
                                rhs=qt_sb[
                                    poff : poff + 64, dblk, j * 512 : (j + 1) * 512
                                ],
                                start=True,
                                stop=True,
                            )
                        eb = e_pool.tile(
                            [128, 1024], BF16, tag="eb", name=f"eb_{j}_{h}_{ib}"
                        )
                        d1 = 2 * ib + 1 - 4 * j
                        if d1 < 0:
                            nc.scalar.activation(eb[:], ps[:], EXP, scale=0.125)
                        else:
                            for t in range(2):
                                i = 2 * ib + t
                                dd = i - 4 * j
                                lo = t * 512
                                if dd < 0:
                                    nc.scalar.activation(
                                        eb[:, lo : lo + 512],
                                        ps[:, lo : lo + 512],
                                        EXP,
                                        scale=0.125,
                                    )
                                else:
                                    z = 128 * dd
                                    if z > 0:
                                        nc.vector.memset(eb[:, lo : lo + z], 0.0)
                                    nc.scalar.activation(
                                        eb[:, lo + z : lo + 512],
                                        ps[:, lo + z : lo + 512],
                                        EXP,
                                        scale=0.125,
                                    )
                                    nc.vector.tensor_mul(
                                        eb[:, lo + z : lo + z + 128],
                                        eb[:, lo + z : lo + z + 128],
                                        mask_sb[:],
                                    )
                        bi += 1
                        want = nq * bi // nbatches
                        for t in range(2):
                            i = 2 * ib + t
                            nc.tensor.matmul(
                                pc[:],
                                lhsT=v_sb[:, i, h, :],
                                rhs=eb[:, t * 512 : (t + 1) * 512],
                                start=(i == 0),
                                stop=(i == nkb - 1),
                            )
                            # a filler quantum between same-bank ctx matmuls
                            # hides the PSUM accumulate turnaround
                            if t == 0 and drained < want:
                                filler[drained]()
                                drained += 1
                        while drained < want:
                            filler[drained]()
                            drained += 1
                    # pull ctx + denominator out of PSUM (DVE only), free pc
                    dn = r_pool.tile([1, 512], F32, tag="dn", bufs=3)
                    nc.vector.tensor_copy(dn[:], pc[64:65, :])
                    rc32 = r_pool.tile([1, 512], F32, tag="rc32", bufs=3)
                    nc.vector.reciprocal_approx_fast(rc32[:], dn[:])
                    rc = r_pool.tile([1, 512], BF16, tag="rc", bufs=4)
                    nc.vector.tensor_copy(rc[:], rc32[:])
                    cu = cu_pool.tile([64, 512], BF16, tag="cu")
                    nc.vector.tensor_copy(cu[:], pc[0:64, :])
                    pending.append((h, cu, rc))
                while drained < nq:
                    filler[drained]()
                    drained += 1
                return [
                    (lambda ent=ent: finish_norm(*ent)) for ent in pending
                ]

            def phase_c_quanta(n):
                quanta = []
                for qq in range(4 * n, 4 * n + 4):
                    for e2 in range(2):

                        def unit(qq=qq, e2=e2):
                            po = psA.tile(
                                [128, 512], F32, tag="ps", name=f"po_{qq}_{e2}"
                            )
                            for p in range(NDB):
                                nc.tensor.matmul(
                                    po[:],
                                    lhsT=ct_sb[:, p, qq * 128 : (qq + 1) * 128],
                                    rhs=wo_sb[:, p, e2 * 512 : (e2 + 1) * 512],
                                    start=(p == 0),
                                    stop=(p == NDB - 1),
                                )
                            ob = o_pool.tile(
                                [128, 512], F32, tag="ob", name=f"ob_{qq}_{e2}"
                            )
                            nc.vector.tensor_copy(ob[:], po[:])
                            nc.sync.dma_start(
                                out=out[
                                    qq * 128 : (qq + 1) * 128,
                                    e2 * 512 : (e2 + 1) * 512,
                                ],
                                in_=ob[:],
                            )

                        quanta.append(unit)
                return quanta

            # ---- main schedule ----
            # A(0) runs plain; B(n) is interleaved with the previous block's
            # leftover normalizations + C(n-1) + A(n+1).
            for q in phase_a_quanta(0):
                q()
            leftover = []
            for n in range(NQB):
                filler = list(leftover)
                if n >= 1:
                    filler += phase_c_quanta(n - 1)
                if n + 1 < NQB:
                    load_xt(n + 1)
                    filler += phase_a_quanta(n + 1)
                leftover = phase_b(n, filler)
            for q in leftover:
                q()
            for q in phase_c_quanta(NQB - 1):
                q()
    nc.compile()
    return nc


_NC_CACHE = None


def _get_nc():
    global _NC_CACHE
    if _NC_CACHE is None:
        _NC_CACHE = build_nc()
    return _NC_CACHE


def make_in_maps(x, Wq, Wk, Wv, Wo):
    x = np.asarray(x, dtype=np.float32).astype(NP_BF16)
    Wq = np.asarray(Wq, dtype=np.float32).astype(NP_BF16)
    Wk = np.asarray(Wk, dtype=np.float32).astype(NP_BF16)
    Wv = np.asarray(Wv, dtype=np.float32).astype(NP_BF16)
    Wo = np.asarray(Wo, dtype=np.float32).astype(NP_BF16)
    in_maps = []
    for c in range(NCORES):
        b, g = c // 2, c % 2
        sl = slice(g * DG, (g + 1) * DG)
        in_maps.append(
            {
                "xt": np.ascontiguousarray(x[b].T),
                "wq": np.ascontiguousarray(Wq[:, sl]),
                "wk": np.ascontiguousarray(Wk[:, sl]),
                "wv": np.ascontiguousarray(Wv[:, sl]),
                "wo": np.ascontiguousarray(Wo[sl, :]),
            }
        )
    return in_maps


def _install_ntff_hook():
    """Shim antenv.axon_hooks (absent in this image) so trace=True works."""
    import sys
    import types

    try:
        import antenv.axon_hooks  # noqa: F401

        return
    except ImportError:
        pass
    try:
        import antenv
        from trn_agent_boot.trn_boot import _ntff_profile_via_ctypes

        hook = _ntff_profile_via_ctypes("/opt/axon/libaxon_pjrt.so")
        mod = types.ModuleType("antenv.axon_hooks")
        mod._hook = hook
        mod.get_axon_ntff_profile_hook = lambda: mod._hook
        mod.set_axon_ntff_profile_hook = lambda h: setattr(mod, "_hook", h)
        sys.modules["antenv.axon_hooks"] = mod
        antenv.axon_hooks = mod
    except Exception as e:  # degrade to no-trace
        print("ntff hook shim failed:", e)


def kernel(x, Wq, Wk, Wv, Wo, bo, _trace=False):
    global LAST_EXEC_TIME_NS
    if _trace:
        _install_ntff_hook()
    bo = np.asarray(bo, dtype=np.float32)
    nc = _get_nc()
    in_maps = make_in_maps(x, Wq, Wk, Wv, Wo)
    res = run_bass_kernel_spmd(nc, in_maps, list(range(NCORES)), trace=_trace)
    LAST_EXEC_TIME_NS = res.exec_time_ns
    out = np.empty((B, S, DOUT), dtype=np.float32)
    for b in range(B):
        out[b] = res.results[2 * b]["out"] + res.results[2 * b + 1]["out"] + bo
    return out


# revision 27
# speedup vs baseline: 1.2347x; 1.0057x over previous
"""Multi-head causal attention (B=4, S=2048, D=1024, H=16) on 8 trn2 NeuronCores.

Sharding: data-parallel over batch (4) x tensor-parallel over heads (2 groups
of 8 heads).  Core c handles batch c//2, head-group c%2.  Each core computes
its 512-wide slice of Q/K/V, causal attention for its 8 heads, and a partial
out-projection (row-parallel Wo).  The host sums the two partials per batch
and adds the bias (the "all-reduce" of the row-parallel out_proj).

Kernel layout notes (per core):
 - x arrives pre-transposed (and pre-cast to bf16) from host as xt
   [1024, 2048] so the contraction dim (d_in) is on partitions for all
   projection matmuls.
 - Q^T, K^T stored [d'=128 (2 heads), s] in bf16: directly usable as
   scores-matmul operands (S^T[k,q] = K^T_tile.T @ Q^T) with d on partitions.
 - V stored naturally [s, d'] with a ones-column appended per head (65-wide
   head slots) so the ctx matmul also produces the softmax denominators.
 - Scores are computed transposed (S^T: k on partitions, q free).  Softmax
   needs no max-stabilization (scores ~ N(0,1) after the 1/8 scale).  Causal
   masking: the fully-masked strip of a diagonal tile is memset to 0, the
   128x128 boundary block is multiplied by a precomputed triangular bf16
   mask, and only the live strip is exp'd.
 - Normalization is kept OFF the PE critical path: per head, the
   unnormalized ctx^T and the denominator row are copied out of PSUM by DVE
   (freeing the PSUM accumulator early); the reciprocal (DVE approx-fast,
   SBUF source only -- PSUM source is broken on HW), the K=1 broadcast
   matmuls, and the final normalize-multiplies are all batched at the end
   of the q-block where they pipeline behind other work.
 - The PE instruction queue is in-order, so phase overlap must be done at
   emission time: the projection matmuls for q-block n+1 are emitted as
   small "filler" quanta interleaved between the attention batches of
   q-block n.  While attention waits on ScalarE exp, PE executes projection
   quanta instead of idling (also keeping the HAM clock-gate warm).
"""

import numpy as np

import concourse.bacc as bacc
import concourse.mybir as mybir
from concourse import tile
from concourse.bass_utils import run_bass_kernel_spmd

F32 = mybir.dt.float32
BF16 = mybir.dt.bfloat16
EXP = mybir.ActivationFunctionType.Exp

B, S, DIN, DOUT, H = 4, 2048, 1024, 1024, 16
NCORES = 8
DG = 512          # d_out slice per core (8 heads)
NH = 8            # heads per core
HD = 64
NKT = DIN // 128  # 8 contraction tiles for projections
NQB = S // 512    # 4 q blocks of 512
NKB = S // 128    # 16 k blocks of 128
NDB = DG // 128   # 4 d'-blocks of 128 (2 heads each)

NP_BF16 = mybir.dt.np(BF16)

LAST_EXEC_TIME_NS = None


def build_nc():
    nc = bacc.Bacc()
    xt = nc.dram_tensor("xt", [DIN, S], BF16, kind="ExternalInput")
    wq = nc.dram_tensor("wq", [DIN, DG], BF16, kind="ExternalInput")
    wk = nc.dram_tensor("wk", [DIN, DG], BF16, kind="ExternalInput")
    wv = nc.dram_tensor("wv", [DIN, DG], BF16, kind="ExternalInput")
    wo = nc.dram_tensor("wo", [DG, DOUT], BF16, kind="ExternalInput")
    out = nc.dram_tensor("out", [S, DOUT], F32, kind="ExternalOutput")

    with tile.TileContext(nc) as tc:
        with (
            tc.tile_pool(name="persist", bufs=1) as persist,
            tc.tile_pool(name="xt", bufs=2) as xt_pool,
            tc.tile_pool(name="eb", bufs=4) as e_pool,
            tc.tile_pool(name="rp", bufs=2) as r_pool,
            tc.tile_pool(name="cu", bufs=9) as cu_pool,
            tc.tile_pool(name="ob", bufs=3) as o_pool,
            tc.tile_pool(name="psA", bufs=3, space="PSUM") as psA,
            tc.tile_pool(name="psC", bufs=2, space="PSUM") as psC,
        ):
            # ---- persistent SBUF tensors ----
            wq_sb = persist.tile([128, NKT, DG], BF16)
            wk_sb = persist.tile([128, NKT, DG], BF16)
            wv_sb = persist.tile([128, NKT, DG], BF16)
            wo_sb = persist.tile([128, NDB, DOUT], BF16)
            qt_sb = persist.tile([128, NDB, S], BF16)
            kt_sb = persist.tile([128, NDB, S], BF16)
            v_sb = persist.tile([128, NKB, NH, HD + 1], BF16)
            ct_sb = persist.tile([128, NDB, S], BF16)
            mask_sb = persist.tile([128, 128], BF16)
            ones_sb = persist.tile([1, 64], BF16)

            # ---- one-time setup ----
            nc.vector.memset(ones_sb[:], 1.0)
            nc.vector.memset(v_sb[:, :, :, HD : HD + 1], 1.0)
            nc.vector.memset(mask_sb[:], 1.0)
            # triangular causal boundary block: keep where q_local >= k_local
            nc.gpsimd.affine_select(
                out=mask_sb[:],
                in_=mask_sb[:],
                pattern=[[1, 128]],
                base=0,
                channel_multiplier=-1,
                compare_op=mybir.AluOpType.is_ge,
                fill=0.0,
            )

            xt_r = xt.rearrange("(kt p) s -> p kt s", p=128)
            xt_tiles = [None] * NQB

            def load_xt(n):
                t = xt_pool.tile([128, NKT, 512], BF16, tag="xt")
                for kt in range(NKT):
                    nc.sync.dma_start(
                        out=t[:, kt, :],
                        in_=xt_r[:, kt, n * 512 : (n + 1) * 512],
                    )
                xt_tiles[n] = t

            # first x block + weights spread over independent DMA queues so
            # the startup transfers run in parallel, not serialized on sync
            load_xt(0)
            for w_dram, w_sb, eng in (
                (wq, wq_sb, nc.gpsimd),
                (wk, wk_sb, nc.scalar),
                (wv, wv_sb, nc.gpsimd),
            ):
                w_r = w_dram.rearrange("(kt p) d -> p kt d", p=128)
                for kt in range(NKT):
                    eng.dma_start(out=w_sb[:, kt, :], in_=w_r[:, kt, :])
            wo_r = wo.rearrange("(t p) e -> p t e", p=128)
            for p in range(NDB):
                nc.scalar.dma_start(out=wo_sb[:, p, :], in_=wo_r[:, p, :])

            def phase_a_quanta(n):
                """Emit projections for s-block n as a list of small closures.

                Each quantum is ~2 matmuls (or one PSUM->SBUF copy) so it can
                be interleaved between attention batches as PE filler.
                """
                quanta = []
                xt_t = xt_tiles[n]
                state = {}

                def q_group(w_sb, dst, mp):
                    def alloc():
                        state[("ps", w_sb.name, mp)] = psA.tile(
                            [128, 1024], F32, tag="ps", name=f"psa_{n}_{w_sb.name}_{mp}"
                        )

                    quanta.append(alloc)
                    for kt in range(NKT):

                        def mm2(kt=kt, w_sb=w_sb, mp=mp):
                            ps = state[("ps", w_sb.name, mp)]
                            for m01 in range(2):  # alternate psum banks
                                m = mp * 2 + m01
                                nc.tensor.matmul(
                                    ps[:, m01 * 512 : (m01 + 1) * 512],
                                    lhsT=w_sb[:, kt, m * 128 : (m + 1) * 128],
                                    rhs=xt_t[:, kt, :],
                                    start=(kt == 0),
                                    stop=(kt == NKT - 1),
                                )

                        quanta.append(mm2)

                    def cp(w_sb=w_sb, dst=dst, mp=mp):
                        ps = state[("ps", w_sb.name, mp)]
                        nc.vector.tensor_copy(
                            dst[:, mp * 2 : mp * 2 + 2, n * 512 : (n + 1) * 512],
                            ps.rearrange("p (m s) -> p m s", m=2),
                        )

                    quanta.append(cp)

                for w_sb, dst in ((wq_sb, qt_sb), (wk_sb, kt_sb)):
                    for mp in range(2):
                        q_group(w_sb, dst, mp)

                def v_group(sp):
                    def alloc(sp=sp):
                        state[("psv", sp)] = psA.tile([128, 1024], F32, tag="ps", name=f"psv_{n}_{sp}")

                    quanta.append(alloc)
                    for kt in range(NKT):

                        def mm2(kt=kt, sp=sp):
                            ps = state[("psv", sp)]
                            for s01 in range(2):  # alternate psum banks
                                ss = sp * 2 + s01
                                nc.tensor.matmul(
                                    ps[:, s01 * 512 : (s01 + 1) * 512],
                                    lhsT=xt_t[:, kt, ss * 128 : (ss + 1) * 128],
                                    rhs=wv_sb[:, kt, :],
                                    start=(kt == 0),
                                    stop=(kt == NKT - 1),
                                )

                        quanta.append(mm2)

                    def cp(sp=sp):
                        ps = state[("psv", sp)]
                        gss = n * 4 + sp * 2
                        nc.vector.tensor_copy(
                            v_sb[:, gss : gss + 2, :, 0:HD],
                            ps.rearrange("p (u h e) -> p u h e", u=2, e=HD),
                        )

                    quanta.append(cp)

                for sp in range(2):
                    v_group(sp)
                return quanta

            def phase_b(j, filler):
                """Attention for q-block j (heads sequential, batches of two
                k-tiles).  `filler` quanta are drained between batches to keep
                PE busy during exp waits."""
                nkb = 4 * j + 4
                nbatches = NH * (nkb // 2)
                nq = len(filler)
                drained = 0
                bi = 0
                pending = []  # (h, cu, rc) awaiting broadcast+multiply

                def finish_norm(h, cu, rc):
                    dblk, poff = h // 2, (h % 2) * 64
                    pb = psA.tile([64, 512], F32, tag="ps", name=f"pb_{j}_{h}")
                    nc.tensor.matmul(
                        pb[:], lhsT=ones_sb[:], rhs=rc[:], start=True, stop=True
                    )
                    rb = r_pool.tile([64, 512], BF16, tag="rb", bufs=2)
                    nc.scalar.copy(rb[:], pb[:])
                    nc.vector.tensor_mul(
                        ct_sb[poff : poff + 64, dblk, j * 512 : (j + 1) * 512],
                        cu[:],
                        rb[:],
                    )

                for h in range(NH):
                    dblk, poff = h // 2, (h % 2) * 64
                    if len(pending) > 1:
                        # norm for head h-2 is ready by now; finishing it here
                        # keeps the broadcast matmul off the critical path
                        finish_norm(*pending.pop(0))
                    pc = psC.tile([65, 512], F32, tag="pc", name=f"pc_{j}_{h}")
                    for ib in range(nkb // 2):
                        ps = psA.tile(
                            [128, 1024], F32, tag="ps", name=f"ps_{j}_{h}_{ib}"
                        )
                        for t in range(2):
                            i = 2 * ib + t
                            nc.tensor.matmul(
                                ps[:, t * 512 : (t + 1) * 512],
                                lhsT=kt_sb[
                                    poff : poff + 64, dblk, i * 128 : (i + 1) * 128
                                ],
                                rhs=qt_sb[
                                    poff : poff + 64, dblk, j * 512 : (j + 1) * 512
                                ],
                                start=True,
                                stop=True,
                            )
                        eb = e_pool.tile(
                            [128, 1024], BF16, tag="eb", name=f"eb_{j}_{h}_{ib}"
                        )
                        d1 = 2 * ib + 1 - 4 * j
                        if d1 < 0:
                            nc.scalar.activation(eb[:], ps[:], EXP, scale=0.125)
                        else:
                            for t in range(2):
                                i = 2 * ib + t
                                dd = i - 4 * j
                                lo = t * 512
                                if dd < 0:
                                    nc.scalar.activation(
                                        eb[:, lo : lo + 512],
                                        ps[:, lo : lo + 512],
                                        EXP,
                                        scale=0.125,
                                    )
                                else:
                                    z = 128 * dd
                                    if z > 0:
                                        nc.vector.memset(eb[:, lo : lo + z], 0.0)
                                    nc.scalar.activation(
                                        eb[:, lo + z : lo + 512],
                                        ps[:, lo + z : lo + 512],
                                        EXP,
                                        scale=0.125,
                                    )
                                    nc.vector.tensor_mul(
                                        eb[:, lo + z : lo + z + 128],
                                        eb[:, lo + z : lo + z + 128],
                                        mask_sb[:],
                                    )
                        bi += 1
                        want = nq * bi // nbatches
                        for t in range(2):
                            i = 2 * ib + t
                            nc.tensor.matmul(
                                pc[:],
                                lhsT=v_sb[:, i, h, :],
                                rhs=eb[:, t * 512 : (t + 1) * 512],
                                start=(i == 0),
                                stop=(i == nkb - 1),
                            )
                            # a filler quantum between same-bank ctx matmuls
                            # hides the PSUM accumulate turnaround
                            if t == 0 and drained < want:
                                filler[drained]()
                                drained += 1
                        while drained < want:
                            filler[drained]()
                            drained += 1
                    # pull ctx + denominator out of PSUM (DVE only), free pc
                    dn = r_pool.tile([1, 512], F32, tag="dn", bufs=3)
                    nc.vector.tensor_copy(dn[:], pc[64:65, :])
                    rc32 = r_pool.tile([1, 512], F32, tag="rc32", bufs=3)
                    nc.vector.reciprocal_approx_fast(rc32[:], dn[:])
                    rc = r_pool.tile([1, 512], BF16, tag="rc", bufs=4)
                    nc.vector.tensor_copy(rc[:], rc32[:])
                    cu = cu_pool.tile([64, 512], BF16, tag="cu")
                    nc.vector.tensor_copy(cu[:], pc[0:64, :])
                    pending.append((h, cu, rc))
                while drained < nq:
                    filler[drained]()
                    drained += 1
                return [
                    (lambda ent=ent: finish_norm(*ent)) for ent in pending
                ]

            def phase_c_quanta(n):
                quanta = []
                for qq in range(4 * n, 4 * n + 4):
                    for e2 in range(2):

                        def unit(qq=qq, e2=e2):
                            po = psA.tile(
                                [128, 512], F32, tag="ps", name=f"po_{qq}_{e2}"
                            )
                            for p in range(NDB):
                                nc.tensor.matmul(
                                    po[:],
                                    lhsT=ct_sb[:, p, qq * 128 : (qq + 1) * 128],
                                    rhs=wo_sb[:, p, e2 * 512 : (e2 + 1) * 512],
                                    start=(p == 0),
                                    stop=(p == NDB - 1),
                                )
                            ob = o_pool.tile(
                                [128, 512], F32, tag="ob", name=f"ob_{qq}_{e2}"
                            )
                            nc.vector.tensor_copy(ob[:], po[:])
                            nc.sync.dma_start(
                                out=out[
                                    qq * 128 : (qq + 1) * 128,
                                    e2 * 512 : (e2 + 1) * 512,
                                ],
                                in_=ob[:],
                            )

                        quanta.append(unit)
                return quanta

            # ---- main schedule ----
            # A(0) runs plain; B(n) is interleaved with the previous block's
            # leftover normalizations + C(n-1) + A(n+1).
            for q in phase_a_quanta(0):
                q()
            leftover = []
            for n in range(NQB):
                filler = list(leftover)
                if n >= 1:
                    filler += phase_c_quanta(n - 1)
                if n + 1 < NQB:
                    load_xt(n + 1)
                    filler += phase_a_quanta(n + 1)
                leftover = phase_b(n, filler)
            for q in leftover:
                q()
            for q in phase_c_quanta(NQB - 1):
                q()
    nc.compile()
    return nc


_NC_CACHE = None


def _get_nc():
    global _NC_CACHE
    if _NC_CACHE is None:
        _NC_CACHE = build_nc()
    return _NC_CACHE


def make_in_maps(x, Wq, Wk, Wv, Wo):
    x = np.asarray(x, dtype=np.float32).astype(NP_BF16)
    Wq = np.asarray(Wq, dtype=np.float32).astype(NP_BF16)
    Wk = np.asarray(Wk, dtype=np.float32).astype(NP_BF16)
    Wv = np.asarray(Wv, dtype=np.float32).astype(NP_BF16)
    Wo = np.asarray(Wo, dtype=np.float32).astype(NP_BF16)
    in_maps = []
    for c in range(NCORES):
        b, g = c // 2, c % 2
        sl = slice(g * DG, (g + 1) * DG)
        in_maps.append(
            {
                "xt": np.ascontiguousarray(x[b].T),
                "wq": np.ascontiguousarray(Wq[:, sl]),
                "wk": np.ascontiguousarray(Wk[:, sl]),
                "wv": np.ascontiguousarray(Wv[:, sl]),
                "wo": np.ascontiguousarray(Wo[sl, :]),
            }
        )
    return in_maps


def _install_ntff_hook():
    """Shim antenv.axon_hooks (absent in this image) so trace=True works."""
    import sys
    import types

    try:
        import antenv.axon_hooks  # noqa: F401

        return
    except ImportError:
        pass
    try:
        import antenv
        from trn_agent_boot.trn_boot import _ntff_profile_via_ctypes

        hook = _ntff_profile_via_ctypes("/opt/axon/libaxon_pjrt.so")
        mod = types.ModuleType("antenv.axon_hooks")
        mod._hook = hook
        mod.get_axon_ntff_profile_hook = lambda: mod._hook
        mod.set_axon_ntff_profile_hook = lambda h: setattr(mod, "_hook", h)
        sys.modules["antenv.axon_hooks"] = mod
        antenv.axon_hooks = mod
    except Exception as e:  # degrade to no-trace
        print("ntff hook shim failed:", e)


def kernel(x, Wq, Wk, Wv, Wo, bo, _trace=False):
    global LAST_EXEC_TIME_NS
    if _trace:
        _install_ntff_hook()
    bo = np.asarray(bo, dtype=np.float32)
    nc = _get_nc()
    in_maps = make_in_maps(x, Wq, Wk, Wv, Wo)
    res = run_bass_kernel_spmd(nc, in_maps, list(range(NCORES)), trace=_trace)
    LAST_EXEC_TIME_NS = res.exec_time_ns
    out = np.empty((B, S, DOUT), dtype=np.float32)
    for b in range(B):
        out[b] = res.results[2 * b]["out"] + res.results[2 * b + 1]["out"] + bo
    return out
